# revision 2
# baseline (speedup 1.0000x reference)
"""Trainium2 Bass kernel for nn_EvidentialGSL (8-core row-sharded), v2.

Same algorithm as the baseline kernel (see kernel.py docstring) with the
hot-loop restructured around DMA-issue cost and SBUF reuse:
  - every multi-tile load/store is a single 2/3-dim-AP DMA (HWDGE fixed
    overhead is ~625ns per dma_start, independent of size)
  - phase A processes i-tiles in pairs so the X^T hi/lo stream is read
    4x instead of 8x; A rows are DMA'd straight into the stripe and the
    relu(S) term is accumulated in place
  - V0T spill tiles are grouped [m=s%8][d=s//8] so phase B can fetch a
    strided octet (all j-tiles = o mod 8) with one DMA and start as soon
    as AllToAll #o lands
  - phase B element-wise work is split across DVE (compare/mult) and
    GpSimd (mask max)
"""
import os
import numpy as np
from contextlib import ExitStack

KPHASE = int(os.environ.get("KPHASE", "3"))

import ml_dtypes
from concourse import bass, bacc, tile, mybir
from concourse.bass_utils import run_bass_kernel_spmd

dt = mybir.dt
AF = mybir.ActivationFunctionType
ALU = mybir.AluOpType

N, D = 8192, 768
H1, H2 = 512, 256
NCORE = 8
P = N // NCORE          # 1024 rows per core
NIT = P // 128          # 8 i-tiles per core
NJT = N // 128          # 64 j-tiles
KD = D // 128           # 6
KH1 = H1 // 128         # 4
KH2 = H2 // 128         # 2
JC = 512                # phase-A j chunk
NJC = N // JC           # 16
NPAIR = NIT // 2        # 4 passes of i-tile pairs


def _softplus(nc, pool, out_ap, in_ap, shp, neg=False):
    t1 = pool.tile(shp, dt.float32, tag="sp_a")
    t2 = pool.tile(shp, dt.float32, tag="sp_b")
    nc.scalar.activation(t1[:], in_ap, AF.Abs)
    nc.scalar.activation(t1[:], t1[:], AF.Exp, scale=-1.0)
    nc.scalar.activation(t1[:], t1[:], AF.Ln, bias=1.0)
    nc.scalar.activation(t2[:], in_ap, AF.Relu, scale=(-1.0 if neg else 1.0))
    nc.vector.tensor_add(out_ap, t1[:], t2[:])


def _sigmoid(nc, pool, out_ap, in_ap, shp):
    t3 = pool.tile(shp, dt.float32, tag="sp_c")
    _softplus(nc, pool, t3[:], in_ap, shp, neg=True)
    nc.scalar.activation(out_ap, t3[:], AF.Exp, scale=-1.0)


def _head(nc, tc, psum, w1sb, b1sb, w2sb, b2sb, whsb, bhsb, xin, out_dram,
          obase, want_u0, hpool, addv):
    """Transposed NIG head on xin [128, KD, P] float32r; writes 4 output rows.

    Head matmuls run in f32r (1 cyc/row vs fp32's 4): ~1.6e-4 relative on the
    head outputs, well inside the 2e-2 budget and with no top-k sensitivity.
    """
    h1 = hpool.tile([128, KH1, P], dt.float32r, tag="h1t")
    for m in range(KH1):
        ps = psum.tile([128, P], dt.float32, tag="ph")
        for h in range(2):
            for k in range(KD):
                nc.tensor.matmul(ps[:, h * 512:(h + 1) * 512],
                                 w1sb[:, k, m * 128:(m + 1) * 128],
                                 xin[:, k, h * 512:(h + 1) * 512],
                                 start=(k == 0), stop=(k == KD - 1))
        nc.scalar.activation(h1[:, m, :], ps[:], AF.Gelu, bias=b1sb[:, m:m + 1])
    h2 = hpool.tile([128, KH2, P], dt.float32r, tag="h2t")
    for m in range(KH2):
        ps = psum.tile([128, P], dt.float32, tag="ph")
        for h in range(2):
            for k in range(KH1):
                nc.tensor.matmul(ps[:, h * 512:(h + 1) * 512],
                                 w2sb[:, k, m * 128:(m + 1) * 128],
                                 h1[:, k, h * 512:(h + 1) * 512],
                                 start=(k == 0), stop=(k == KH1 - 1))
        nc.scalar.activation(h2[:, m, :], ps[:], AF.Gelu, bias=b2sb[:, m:m + 1])
    ps4 = psum.tile([4, P], dt.float32, tag="ph")
    for h in range(2):
        for k in range(KH2):
            nc.tensor.matmul(ps4[:, h * 512:(h + 1) * 512], whsb[:, k, 0:4],
                             h2[:, k, h * 512:(h + 1) * 512],
                             start=(k == 0), stop=(k == KH2 - 1))
    r4 = hpool.tile([4, P], dt.float32, tag="r4")
    nc.scalar.activation(r4[:], ps4[:], AF.Identity, bias=bhsb[0:4, 0:1])
    nc.sync.dma_start(out=out_dram[obase:obase + 1, :], in_=r4[0:1, :])
    o1 = hpool.tile([4, P], dt.float32, tag="o4")
    _softplus(nc, hpool, o1[:], r4[:], [4, P])
    nc.vector.tensor_scalar(o1[:], o1[:], addv[0:4, 0:1], None, ALU.add)
    nc.sync.dma_start(out=out_dram[obase + 1:obase + 4, :], in_=o1[1:4, :])
    if not want_u0:
        return None
    a0t = hpool.tile([1, P], dt.float32, tag="a0t")
    b0t = hpool.tile([1, P], dt.float32, tag="b0t")
    nc.sync.dma_start(out=a0t[:], in_=o1[2:3, :])
    nc.sync.dma_start(out=b0t[:], in_=o1[3:4, :])
    nc.vector.tensor_scalar(a0t[:], a0t[:], -1.0, 1e-8, ALU.add, ALU.max)
    nc.vector.reciprocal(a0t[:], a0t[:])
    u0 = hpool.tile([1, P], dt.float32, tag="u0")
    nc.vector.tensor_mul(u0[:], b0t[:], a0t[:])
    return u0


def build_nc(beta: float, gam: float, eps2: float):
    nc = bacc.Bacc("TRN2", target_bir_lowering=False, debug=False,
                   num_devices=NCORE)
    f32, f32r, bf16, u8 = dt.float32, dt.float32r, dt.bfloat16, dt.uint8

    # X reshaped [a][b][128][D] with j-tile jt = a*8 + b, so a strided octet
    # (fixed b) is a single 3D-AP DMA.
    X_d = nc.dram_tensor("X", [NIT, NCORE, 128, D], f32, kind="ExternalInput").ap()
    XTHI_d = nc.dram_tensor("XTHI", [KD, 128, N], bf16, kind="ExternalInput").ap()
    XTLO_d = nc.dram_tensor("XTLO", [KD, 128, N], bf16, kind="ExternalInput").ap()
    XMYT_d = nc.dram_tensor("XMYT", [KD, 128, P], f32, kind="ExternalInput").ap()
    AROW_d = nc.dram_tensor("AROW", [P, N], f32, kind="ExternalInput").ap()
    W_d = nc.dram_tensor("W_gm", [KD, 128, D], f32, kind="ExternalInput").ap()
    ihw1_d = nc.dram_tensor("ih_w1", [KD, 128, H1], f32, kind="ExternalInput").ap()
    ihb1_d = nc.dram_tensor("ih_b1", [KH1, 128], f32, kind="ExternalInput").ap()
    ihw2_d = nc.dram_tensor("ih_w2", [KH1, 128, H2], f32, kind="ExternalInput").ap()
    ihb2_d = nc.dram_tensor("ih_b2", [KH2, 128], f32, kind="ExternalInput").ap()
    ihwh_d = nc.dram_tensor("ih_wh", [KH2, 128, 4], f32, kind="ExternalInput").ap()
    ihbh_d = nc.dram_tensor("ih_bh", [4], f32, kind="ExternalInput").ap()
    gcnw_d = nc.dram_tensor("gcn_w", [KD, 128, D], f32, kind="ExternalInput").ap()
    gcnb_d = nc.dram_tensor("gcn_b", [KD, 128], f32, kind="ExternalInput").ap()
    fhw1_d = nc.dram_tensor("fh_w1", [KD, 128, H1], f32, kind="ExternalInput").ap()
    fhb1_d = nc.dram_tensor("fh_b1", [KH1, 128], f32, kind="ExternalInput").ap()
    fhw2_d = nc.dram_tensor("fh_w2", [KH1, 128, H2], f32, kind="ExternalInput").ap()
    fhb2_d = nc.dram_tensor("fh_b2", [KH2, 128], f32, kind="ExternalInput").ap()
    fhwh_d = nc.dram_tensor("fh_wh", [KH2, 128, 4], f32, kind="ExternalInput").ap()
    fhbh_d = nc.dram_tensor("fh_bh", [4], f32, kind="ExternalInput").ap()

    OUT_d = nc.dram_tensor("OUT", [8, P], f32, kind="ExternalOutput").ap()

    pid = nc.partition_id()
    groups = [list(range(NCORE))]

    with tile.TileContext(nc) as tc, ExitStack() as top:
        const = top.enter_context(tc.tile_pool(name="const", bufs=1))
        dram = top.enter_context(tc.tile_pool(name="dram", bufs=1, space="DRAM"))

        # V0T spill grouped [m = s%8][d = s//8][128][P]: consecutive-s write
        # batches are one 3D AP, strided-octet reads are one 3D AP.
        V0T_t = dram.tile([8, NIT, 128, P], f32)
        RSEND_t = dram.tile([NIT, NCORE, 128, P], u8)
        RRECV_t = dram.tile([NIT, NCORE, 128, P], u8)
        TMY_t = dram.tile([NIT, 128], f32)
        GD_t = dram.tile([1, P], f32)
        GALL_t = dram.tile([NCORE, P], f32)

        # ---- constants
        iota_i = const.tile([128, 128], dt.int32)
        nc.gpsimd.iota(iota_i[:], pattern=[[1, 128]], base=0, channel_multiplier=0)
        pidx_i = const.tile([128, 1], dt.int32)
        nc.gpsimd.iota(pidx_i[:], pattern=[[0, 1]], base=0, channel_multiplier=1)
        iota_f = const.tile([128, 128], f32)
        nc.vector.tensor_copy(iota_f[:], iota_i[:])
        pidx_f = const.tile([128, 1], f32)
        nc.vector.tensor_copy(pidx_f[:], pidx_i[:])
        eye = const.tile([128, 128], f32)
        nc.vector.tensor_scalar(eye[:], iota_f[:], pidx_f[:, 0:1], None, ALU.is_equal)
        ident = const.tile([128, 128], f32)
        nc.vector.tensor_copy(ident[:], eye[:])
        ones1 = const.tile([1, 128], f32)
        nc.vector.memset(ones1[:], 1.0)
        ones_f = const.tile([128, 1], f32)
        nc.vector.memset(ones_f[:], 1.0)
        ones_r = const.tile([128, 1], f32r)
        nc.vector.tensor_copy(ones_r[:], ones_f[:])
        addv = const.tile([128, 1], f32)
        nc.vector.tensor_scalar(addv[:], pidx_f[:], 2.0, None, ALU.is_equal)
        nc.vector.tensor_scalar(addv[:], addv[:], 1.0, 1e-6, ALU.mult, ALU.add)

        def load_kmaj(pool, dram_ap, kt, cols, dtype=f32, tag=None):
            t = pool.tile([128, kt, cols], dtype, tag=tag or f"w_{dram_ap.tensor.name}")
            nc.sync.dma_start(out=t[:], in_=dram_ap[:, :, :].bitcast(dtype)
                              .rearrange("k p c -> p k c"))
            return t

        def load_bias(pool, dram_ap, kt):
            tg = f"b_{dram_ap.tensor.name}"
            if kt == 0:
                t = pool.tile([4, 1], f32, tag=tg)
                nc.sync.dma_start(out=t[:, 0:1], in_=dram_ap[0:4])
            else:
                t = pool.tile([128, kt], f32, tag=tg)
                nc.sync.dma_start(out=t[:], in_=dram_ap[:, :].rearrange("k p -> p k"))
            return t

        t2rep = const.tile([128, P], f32)

        # ================= early phase: XWT, head1, G =================
        xw_stack = ExitStack()
        xwP = xw_stack.enter_context(tc.tile_pool(name="xwP", bufs=1))
        xwhi = xwP.tile([128, KD, P], bf16, tag="xwhi")
        xwlo = xwP.tile([128, KD, P], bf16, tag="xwlo")
        with tc.tile_pool(name="early", bufs=1) as early, \
             tc.tile_pool(name="psE", bufs=1, space="PSUM") as psE:
            xmyt = early.tile([128, KD, P], f32)
            nc.sync.dma_start(out=xmyt[:],
                              in_=XMYT_d[:, :, :].rearrange("k p c -> p k c"))
            Wsb = load_kmaj(early, W_d, KD, D)
            for m in range(KD):
                ps = psE.tile([128, P], f32, tag="pxw")
                for h in range(2):
                    for k in range(KD):
                        nc.tensor.matmul(ps[:, h * 512:(h + 1) * 512],
                                         Wsb[:, k, m * 128:(m + 1) * 128],
                                         xmyt[:, k, h * 512:(h + 1) * 512],
                                         start=(k == 0), stop=(k == KD - 1))
                nc.scalar.activation(xwhi[:, m, :], ps[:], AF.Copy)
                nc.vector.tensor_sub(xwlo[:, m, :], ps[:], xwhi[:, m, :])

            ihw1 = load_kmaj(early, ihw1_d, KD, H1, dt.float32r)
            ihw2 = load_kmaj(early, ihw2_d, KH1, H2, dt.float32r)
            ihwh = load_kmaj(early, ihwh_d, KH2, 4, dt.float32r)
            xmyt_r = early.tile([128, KD, P], dt.float32r, tag="xmyt_r")
            nc.sync.dma_start(out=xmyt_r[:],
                              in_=XMYT_d[:, :, :].bitcast(dt.float32r)
                              .rearrange("k p c -> p k c"))
            ihb1 = load_bias(early, ihb1_d, KH1)
            ihb2 = load_bias(early, ihb2_d, KH2)
            ihbh = load_bias(early, ihbh_d, 0)
            with tc.tile_pool(name="hpool", bufs=1) as hpool, \
                 tc.tile_pool(name="psE2", bufs=2, space="PSUM") as psE2:
                u0 = _head(nc, tc, psE2, ihw1, ihb1, ihw2, ihb2, ihwh, ihbh,
                           xmyt_r, OUT_d, 0, True, hpool, addv)
                sg = hpool.tile([1, P], f32, tag="sg")
                _sigmoid(nc, hpool, sg[:], u0[:], [1, P])
                gmy = hpool.tile([1, P], f32, tag="gmy")
                nc.vector.tensor_scalar(gmy[:], sg[:], float(np.float32(-gam)),
                                        1.0, ALU.mult, ALU.add)
                nc.sync.dma_start(out=GD_t[0:1, :], in_=gmy[0:1, :])
                nc.gpsimd.collective_compute("AllGather", ALU.bypass,
                                             replica_groups=groups,
                                             ins=[GD_t.opt()], outs=[GALL_t.opt()])

        # ================= phase A =================
        NPAIR_RUN = NPAIR if KPHASE != 0 else 1
        with tc.tile_pool(name="stripeP", bufs=3) as stripeP, \
             tc.tile_pool(name="pa", bufs=2) as pa, \
             tc.tile_pool(name="pam", bufs=2) as pam, \
             tc.tile_pool(name="pam1", bufs=1) as pam1, \
             tc.tile_pool(name="psA", bufs=2, space="PSUM") as psA, \
             tc.tile_pool(name="psT", bufs=4, space="PSUM") as psT:
            for pr in range(NPAIR_RUN):
                stripes = []
                for i01 in range(2):
                    it = pr * 2 + i01
                    st = stripeP.tile([128, N], f32, tag="v0")
                    nc.sync.dma_start(out=st[:], in_=AROW_d[it * 128:(it + 1) * 128, :])
                    stripes.append(st)
                for jc in range(NJC):
                    xh = pa.tile([128, KD, JC], bf16, tag="xth")
                    xl = pa.tile([128, KD, JC], bf16, tag="xtl")
                    nc.sync.dma_start(
                        out=xh[:], in_=XTHI_d[:, :, jc * JC:(jc + 1) * JC]
                        .rearrange("k p c -> p k c"))
                    nc.sync.dma_start(
                        out=xl[:], in_=XTLO_d[:, :, jc * JC:(jc + 1) * JC]
                        .rearrange("k p c -> p k c"))
                    for i01 in range(2):
                        it = pr * 2 + i01
                        ps = psA.tile([128, JC], f32, tag=f"psv{i01}")
                        first = True
                        for pi, (aa, bb) in enumerate(
                                ((xwhi, xh), (xwhi, xl), (xwlo, xh))):
                            for k in range(KD):
                                nc.tensor.matmul(
                                    ps[:], aa[:, k, it * 128:(it + 1) * 128],
                                    bb[:, k, :],
                                    start=first, stop=(pi == 2 and k == KD - 1))
                                first = False
                        rel = pa.tile([128, JC], f32, tag=f"rel{i01}")
                        nc.scalar.activation(rel[:], ps[:], AF.Relu,
                                             scale=float(np.float32(1.0 / beta)))
                        sl = stripes[i01][:, jc * JC:(jc + 1) * JC]
                        nc.gpsimd.tensor_add(sl, sl, rel[:])
                for i01 in range(2):
                    it = pr * 2 + i01
                    stripe = stripes[i01]
                    top8 = pam.tile([128, 8], f32, tag="top8")
                    nc.vector.max(top8[:], stripe[:])
                    nc.sync.dma_start(out=TMY_t[it:it + 1, :], in_=top8[:, 4:5])
                    off = nc.snap(pid * P + it * 128, min_val=0, max_val=N - 128)
                    dsub = stripe[:, bass.ds(off, 128)]
                    nc.vector.scalar_tensor_tensor(dsub, eye[:], -1e9, dsub,
                                                   ALU.mult, ALU.add)
                    rmask = pam1.tile([128, N], u8, tag="rmask")
                    nc.vector.tensor_scalar(rmask[:], stripe[:], top8[:, 4:5], None,
                                            ALU.is_ge)
                    nc.sync.dma_start(
                        out=RSEND_t[it].rearrange("c p j -> p c j"), in_=rmask[:])
                    for d8 in range(NIT):
                        ct = pa.tile([128, 8, 128], f32, tag="ctr")
                        for m8 in range(8):
                            s = d8 * 8 + m8
                            pst = psT.tile([128, 128], f32, tag="ptr")
                            nc.tensor.transpose(pst[:], stripe[:, s * 128:(s + 1) * 128],
                                                ident[:])
                            nc.scalar.activation(ct[:, m8, :], pst[:], AF.Copy)
                        nc.sync.dma_start(
                            out=V0T_t[:, d8, :, it * 128:(it + 1) * 128]
                            .rearrange("m p c -> p m c"),
                            in_=ct[:])
                    nc.gpsimd.collective_compute(
                        "AllToAll", ALU.bypass, replica_groups=groups,
                        ins=[RSEND_t[it].opt()], outs=[RRECV_t[it].opt()])

        # T2rep broadcast (exact fp32 K=1 matmul)
        trow = const.tile([1, P], f32)
        nc.sync.dma_start(out=trow[0:1, :], in_=TMY_t[:])
        if KPHASE >= 2:
          with tc.tile_pool(name="psB1", bufs=1, space="PSUM") as psB1:
            for h in range(2):
                psb = psB1.tile([128, 512], f32, tag="pbc")
                nc.tensor.matmul(psb[:], ones1[:], trow[0:1, h * 512:(h + 1) * 512],
                                 start=True, stop=True)
                nc.scalar.activation(t2rep[:, h * 512:(h + 1) * 512], psb[:], AF.Copy)

        # ================= phase B =================
        xw_stack.close()
        if KPHASE >= 2:
            bc = top.enter_context(tc.tile_pool(name="bc", bufs=1))
            pt_acc = bc.tile([128, KD, P], f32, tag="pt_acc")
            rs_acc = bc.tile([1, P], f32, tag="rs_acc")
            gcnw = load_kmaj(bc, gcnw_d, KD, D, f32r)
            gcnb = load_bias(bc, gcnb_d, KD)
            with tc.tile_pool(name="pb", bufs=1) as pb, \
                 tc.tile_pool(name="pbm", bufs=2) as pbm, \
                 tc.tile_pool(name="pbt", bufs=1) as pbt, \
                 tc.tile_pool(name="agtP", bufs=1) as agtP, \
                 tc.tile_pool(name="psP", bufs=1, space="PSUM") as psP, \
                 tc.tile_pool(name="psR", bufs=1, space="PSUM") as psR:
                for o in range(8):
                    # strided octet: j-tiles jt = o + 8*l for l = 0..7.
                    # bufs=1 pools with per-half tags: octet o+1's first-half
                    # load overlaps octet o's second-half compute.
                    v0t, xt_, rcv = [], [], []
                    for half in range(2):
                        vt = pb.tile([128, 4, P], f32, tag=f"v0t{half}")
                        nc.sync.dma_start(
                            out=vt[:],
                            in_=V0T_t[o, half * 4:(half + 1) * 4]
                            .rearrange("d p c -> p d c"))
                        v0t.append(vt)
                        rc = pb.tile([128, 4, P], u8, tag=f"rcv{half}")
                        nc.sync.dma_start(
                            out=rc[:],
                            in_=RRECV_t[o, half * 4:(half + 1) * 4]
                            .rearrange("c p i -> p c i"))
                        rcv.append(rc)
                        xt = pb.tile([128, 4, D], f32, tag=f"xrow{half}")
                        nc.sync.dma_start(
                            out=xt[:],
                            in_=X_d[half * 4:(half + 1) * 4, o]
                            .rearrange("a p d -> p a d"))
                        xt_.append(xt)
                    gsl = pbm.tile([128, NCORE], f32, tag="gsl")
                    nc.sync.dma_start(
                        out=gsl[:],
                        in_=GALL_t[:, o * 128:(o + 1) * 128].rearrange("l p -> p l"))
                    agts, xgs = [], []
                    for l in range(8):
                        vt = v0t[l // 4][:, l % 4, :]
                        mlt = pbm.tile([128, P], bf16, tag="mlt")
                        nc.vector.tensor_tensor(mlt[:], vt, t2rep[:], ALU.is_ge)
                        msk = pbm.tile([128, P], bf16, tag="msk")
                        nc.vector.tensor_tensor(msk[:], mlt[:],
                                                rcv[l // 4][:, l % 4, :], ALU.max)
                        agt = agtP.tile([128, P], f32r, tag=f"agt{l}")
                        nc.vector.tensor_tensor(agt[:], vt, msk[:], ALU.mult)
                        agts.append(agt)
                        xg = agtP.tile([128, D], f32r, tag=f"xg{l}")
                        nc.scalar.activation(xg[:], xt_[l // 4][:, l % 4, :],
                                             AF.Copy, scale=gsl[:, l:l + 1])
                        xgs.append(xg)
                    for h in range(2):
                        pp = psP.tile([128, KD, 512], f32, tag="pp")
                        for l in range(8):
                            for m in range(KD):
                                nc.tensor.matmul(pp[:, m, :],
                                                 xgs[l][:, m * 128:(m + 1) * 128],
                                                 agts[l][:, h * 512:(h + 1) * 512],
                                                 start=(l == 0), stop=(l == 7))
                        for m in range(KD):
                            if o == 0:
                                nc.vector.tensor_copy(
                                    pt_acc[:, m, h * 512:(h + 1) * 512], pp[:, m, :])
                            else:
                                nc.vector.tensor_add(
                                    pt_acc[:, m, h * 512:(h + 1) * 512],
                                    pt_acc[:, m, h * 512:(h + 1) * 512], pp[:, m, :])
                    for h in range(2):
                        pr2 = psR.tile([1, 512], f32, tag="pr")
                        for l in range(8):
                            nc.tensor.matmul(pr2[0:1, :],
                                             ones_r[:, 0:1],
                                             agts[l][:, h * 512:(h + 1) * 512],
                                             start=(l == 0), stop=(l == 7))
                        if o == 0:
                            nc.vector.tensor_copy(rs_acc[0:1, h * 512:(h + 1) * 512],
                                                  pr2[:])
                        else:
                            nc.vector.tensor_add(rs_acc[0:1, h * 512:(h + 1) * 512],
                                                 rs_acc[0:1, h * 512:(h + 1) * 512],
                                                 pr2[:])

        # ================= phase C =================
        if KPHASE >= 3:
            with tc.tile_pool(name="pc", bufs=1) as pc, \
                 tc.tile_pool(name="hpool2", bufs=1) as hpool2, \
                 tc.tile_pool(name="psC", bufs=1, space="PSUM") as psC, \
                 tc.tile_pool(name="psCh", bufs=2, space="PSUM") as psCh:
                fhw1 = load_kmaj(pc, fhw1_d, KD, H1, f32r)
                fhw2 = load_kmaj(pc, fhw2_d, KH1, H2, f32r)
                fhwh = load_kmaj(pc, fhwh_d, KH2, 4, f32r)
                fhb1 = load_bias(pc, fhb1_d, KH1)
                fhb2 = load_bias(pc, fhb2_d, KH2)
                fhbh = load_bias(pc, fhbh_d, 0)
                pt_acc_r = pc.tile([128, KD, P], f32r, tag="pt_acc_r")
                nc.vector.tensor_copy(pt_acc_r[:], pt_acc[:])
                dinv = pc.tile([1, P], f32, tag="dinv")
                nc.vector.tensor_scalar(dinv[:], rs_acc[:], float(np.float32(eps2)),
                                        None, ALU.max)
                nc.vector.reciprocal(dinv[:], dinv[:])
                drep = pc.tile([128, P], f32)
                psb = psC.tile([128, P], f32, tag="pxw")
                for h in range(2):
                    nc.tensor.matmul(psb[:, h * 512:(h + 1) * 512], ones1[:],
                                     dinv[0:1, h * 512:(h + 1) * 512],
                                     start=True, stop=True)
                nc.scalar.activation(drep[:], psb[:], AF.Copy)

                xmyt = pc.tile([128, KD, P], f32, tag="xmyt2")
                nc.sync.dma_start(out=xmyt[:],
                                  in_=XMYT_d[:, :, :].rearrange("k p c -> p k c"))

                xpm = pc.tile([128, KD, P], f32r)
                for m in range(KD):
                    ps = psC.tile([128, P], f32, tag="pxw")
                    for h in range(2):
                        for k in range(KD):
                            nc.tensor.matmul(ps[:, h * 512:(h + 1) * 512],
                                             gcnw[:, k, m * 128:(m + 1) * 128],
                                             pt_acc_r[:, k, h * 512:(h + 1) * 512],
                                             start=(k == 0), stop=(k == KD - 1))
                    tmp = pc.tile([128, P], f32, tag="mtmp")
                    nc.vector.tensor_mul(tmp[:], ps[:], drep[:])
                    mf = pc.tile([128, P], f32, tag="mf")
                    nc.scalar.activation(mf[:], tmp[:], AF.Gelu, bias=gcnb[:, m:m + 1])
                    nc.vector.tensor_add(xpm[:, m, :], xmyt[:, m, :], mf[:])

                _head(nc, tc, psCh, fhw1, fhb1, fhw2, fhb2, fhwh, fhbh,
                      xpm, OUT_d, 4, False, hpool2, addv)

    nc.finalize()
    return nc


_NC_CACHE = {}
_last_in_maps = None


def kernel(**inputs) -> tuple:
    X = np.ascontiguousarray(np.asarray(inputs["X"], dtype=np.float32))
    A = np.asarray(inputs["A"], dtype=np.float32)
    ra = float(np.asarray(inputs["ra"], dtype=np.float64))
    gam = float(np.asarray(inputs["gam"], dtype=np.float64))
    al = float(np.float32(1.0) / (np.float32(1.0) + np.float32(np.exp(-np.float32(ra)))))
    beta = al / (1.0 - al)
    eps2 = 1e-8 / al

    XT = np.ascontiguousarray(X.T)
    XTHI = XT.astype(ml_dtypes.bfloat16)
    XTLO = (XT - XTHI.astype(np.float32)).astype(ml_dtypes.bfloat16)

    key = (round(beta, 12), round(gam, 12), KPHASE)
    if key not in _NC_CACHE:
        _NC_CACHE[key] = build_nc(beta, gam, eps2)
    nc = _NC_CACHE[key]

    rep = {
        "X": X.reshape(NIT, NCORE, 128, D),
        "XTHI": XTHI.reshape(KD, 128, N),
        "XTLO": XTLO.reshape(KD, 128, N),
        "W_gm": None, "gcn_w": None,
    }
    for k, kt, cols in (("W_gm", KD, D), ("ih_w1", KD, H1), ("ih_w2", KH1, H2),
                        ("ih_wh", KH2, 4), ("gcn_w", KD, D), ("fh_w1", KD, H1),
                        ("fh_w2", KH1, H2), ("fh_wh", KH2, 4)):
        rep[k] = np.ascontiguousarray(
            np.asarray(inputs[k], dtype=np.float32)).reshape(kt, 128, cols)
    for k, kt in (("ih_b1", KH1), ("ih_b2", KH2), ("gcn_b", KD),
                  ("fh_b1", KH1), ("fh_b2", KH2)):
        rep[k] = np.ascontiguousarray(
            np.asarray(inputs[k], dtype=np.float32)).reshape(kt, 128)
    for k in ("ih_bh", "fh_bh"):
        rep[k] = np.ascontiguousarray(np.asarray(inputs[k], dtype=np.float32))

    in_maps = []
    for c in range(NCORE):
        m = dict(rep)
        m["XMYT"] = np.ascontiguousarray(XT[:, c * P:(c + 1) * P]).reshape(KD, 128, P)
        m["AROW"] = np.ascontiguousarray(A[c * P:(c + 1) * P, :])
        in_maps.append(m)

    global _last_in_maps
    _last_in_maps = in_maps
    res = run_bass_kernel_spmd(nc, in_maps, list(range(NCORE)))
    full = np.concatenate([res.results[c]["OUT"] for c in range(NCORE)], axis=1)
    return tuple(full[i] for i in range(8))


if __name__ == "__main__":
    import jax
    import reference
    cpu = jax.devices("cpu")[0]
    with jax.default_device(cpu):
        inp = reference.setup_inputs()
        inp = {k: np.asarray(v) for k, v in inp.items()}
    got = kernel(**inp)
    with jax.default_device(cpu):
        exp = [np.asarray(x) for x in reference.reference(
            **{k: jax.device_put(v, cpu) for k, v in inp.items()})]
    for i, (g, e) in enumerate(zip(got, exp)):
        e = np.asarray(e)
        err = np.abs(g - e).max()
        rel = err / max(np.abs(e).max(), 1e-9)
        print(f"out{i}: maxabs {err:.3e} rel {rel:.3e}")


# revision 3
# speedup vs baseline: 1.1694x; 1.1694x over previous
"""Trainium2 Bass kernel for nn_EvidentialGSL (8-core row-sharded), v2.

Same algorithm as the baseline kernel (see kernel.py docstring) with the
hot-loop restructured around DMA-issue cost and SBUF reuse:
  - every multi-tile load/store is a single 2/3-dim-AP DMA (HWDGE fixed
    overhead is ~625ns per dma_start, independent of size)
  - phase A processes i-tiles in pairs so the X^T hi/lo stream is read
    4x instead of 8x; A rows are DMA'd straight into the stripe and the
    relu(S) term is accumulated in place
  - V0T spill tiles are grouped [m=s%8][d=s//8] so phase B can fetch a
    strided octet (all j-tiles = o mod 8) with one DMA and start as soon
    as AllToAll #o lands
  - phase B element-wise work is split across DVE (compare/mult) and
    GpSimd (mask max)
"""
import os
import numpy as np
from contextlib import ExitStack

KPHASE = int(os.environ.get("KPHASE", "3"))

import ml_dtypes
from concourse import bass, bacc, tile, mybir
from concourse.bass_utils import run_bass_kernel_spmd

dt = mybir.dt
AF = mybir.ActivationFunctionType
ALU = mybir.AluOpType

N, D = 8192, 768
H1, H2 = 512, 256
NCORE = 8
P = N // NCORE          # 1024 rows per core
NIT = P // 128          # 8 i-tiles per core
NJT = N // 128          # 64 j-tiles
KD = D // 128           # 6
KH1 = H1 // 128         # 4
KH2 = H2 // 128         # 2
JC = 512                # phase-A j chunk
NJC = N // JC           # 16
NPAIR = NIT // 2        # 4 passes of i-tile pairs


def _softplus(nc, pool, out_ap, in_ap, shp, neg=False):
    t1 = pool.tile(shp, dt.float32, tag="sp_a")
    t2 = pool.tile(shp, dt.float32, tag="sp_b")
    nc.scalar.activation(t1[:], in_ap, AF.Abs)
    nc.scalar.activation(t1[:], t1[:], AF.Exp, scale=-1.0)
    nc.scalar.activation(t1[:], t1[:], AF.Ln, bias=1.0)
    nc.scalar.activation(t2[:], in_ap, AF.Relu, scale=(-1.0 if neg else 1.0))
    nc.vector.tensor_add(out_ap, t1[:], t2[:])


def _sigmoid(nc, pool, out_ap, in_ap, shp):
    t3 = pool.tile(shp, dt.float32, tag="sp_c")
    _softplus(nc, pool, t3[:], in_ap, shp, neg=True)
    nc.scalar.activation(out_ap, t3[:], AF.Exp, scale=-1.0)


def _head(nc, tc, psum, w1sb, b1sb, w2sb, b2sb, whsb, bhsb, xin, out_dram,
          obase, want_u0, hpool, addv):
    """Transposed NIG head on xin [128, KD, P] float32r; writes 4 output rows.

    Head matmuls run in f32r (1 cyc/row vs fp32's 4): ~1.6e-4 relative on the
    head outputs, well inside the 2e-2 budget and with no top-k sensitivity.
    """
    h1 = hpool.tile([128, KH1, P], dt.float32r, tag="h1t")
    for m in range(KH1):
        ps = psum.tile([128, P], dt.float32, tag="ph")
        for h in range(2):
            for k in range(KD):
                nc.tensor.matmul(ps[:, h * 512:(h + 1) * 512],
                                 w1sb[:, k, m * 128:(m + 1) * 128],
                                 xin[:, k, h * 512:(h + 1) * 512],
                                 start=(k == 0), stop=(k == KD - 1))
        nc.scalar.activation(h1[:, m, :], ps[:], AF.Gelu, bias=b1sb[:, m:m + 1])
    h2 = hpool.tile([128, KH2, P], dt.float32r, tag="h2t")
    for m in range(KH2):
        ps = psum.tile([128, P], dt.float32, tag="ph")
        for h in range(2):
            for k in range(KH1):
                nc.tensor.matmul(ps[:, h * 512:(h + 1) * 512],
                                 w2sb[:, k, m * 128:(m + 1) * 128],
                                 h1[:, k, h * 512:(h + 1) * 512],
                                 start=(k == 0), stop=(k == KH1 - 1))
        nc.scalar.activation(h2[:, m, :], ps[:], AF.Gelu, bias=b2sb[:, m:m + 1])
    ps4 = psum.tile([4, P], dt.float32, tag="ph")
    for h in range(2):
        for k in range(KH2):
            nc.tensor.matmul(ps4[:, h * 512:(h + 1) * 512], whsb[:, k, 0:4],
                             h2[:, k, h * 512:(h + 1) * 512],
                             start=(k == 0), stop=(k == KH2 - 1))
    r4 = hpool.tile([4, P], dt.float32, tag="r4")
    nc.scalar.activation(r4[:], ps4[:], AF.Identity, bias=bhsb[0:4, 0:1])
    nc.sync.dma_start(out=out_dram[obase:obase + 1, :], in_=r4[0:1, :])
    o1 = hpool.tile([4, P], dt.float32, tag="o4")
    _softplus(nc, hpool, o1[:], r4[:], [4, P])
    nc.vector.tensor_scalar(o1[:], o1[:], addv[0:4, 0:1], None, ALU.add)
    nc.sync.dma_start(out=out_dram[obase + 1:obase + 4, :], in_=o1[1:4, :])
    if not want_u0:
        return None
    a0t = hpool.tile([1, P], dt.float32, tag="a0t")
    b0t = hpool.tile([1, P], dt.float32, tag="b0t")
    nc.sync.dma_start(out=a0t[:], in_=o1[2:3, :])
    nc.sync.dma_start(out=b0t[:], in_=o1[3:4, :])
    nc.vector.tensor_scalar(a0t[:], a0t[:], -1.0, 1e-8, ALU.add, ALU.max)
    nc.vector.reciprocal(a0t[:], a0t[:])
    u0 = hpool.tile([1, P], dt.float32, tag="u0")
    nc.vector.tensor_mul(u0[:], b0t[:], a0t[:])
    return u0


def build_nc(beta: float, gam: float, eps2: float):
    nc = bacc.Bacc("TRN2", target_bir_lowering=False, debug=False,
                   num_devices=NCORE)
    f32, f32r, bf16, u8 = dt.float32, dt.float32r, dt.bfloat16, dt.uint8

    # X reshaped [a][b][128][D] with j-tile jt = a*8 + b, so a strided octet
    # (fixed b) is a single 3D-AP DMA.
    X_d = nc.dram_tensor("X", [NIT, NCORE, 128, D], f32, kind="ExternalInput").ap()
    XTHI_d = nc.dram_tensor("XTHI", [KD, 128, N], bf16, kind="ExternalInput").ap()
    XTLO_d = nc.dram_tensor("XTLO", [KD, 128, N], bf16, kind="ExternalInput").ap()
    XMYT_d = nc.dram_tensor("XMYT", [KD, 128, P], f32, kind="ExternalInput").ap()
    AROW_d = nc.dram_tensor("AROW", [P, N], f32, kind="ExternalInput").ap()
    W_d = nc.dram_tensor("W_gm", [KD, 128, D], f32, kind="ExternalInput").ap()
    ihw1_d = nc.dram_tensor("ih_w1", [KD, 128, H1], f32, kind="ExternalInput").ap()
    ihb1_d = nc.dram_tensor("ih_b1", [KH1, 128], f32, kind="ExternalInput").ap()
    ihw2_d = nc.dram_tensor("ih_w2", [KH1, 128, H2], f32, kind="ExternalInput").ap()
    ihb2_d = nc.dram_tensor("ih_b2", [KH2, 128], f32, kind="ExternalInput").ap()
    ihwh_d = nc.dram_tensor("ih_wh", [KH2, 128, 4], f32, kind="ExternalInput").ap()
    ihbh_d = nc.dram_tensor("ih_bh", [4], f32, kind="ExternalInput").ap()
    gcnw_d = nc.dram_tensor("gcn_w", [KD, 128, D], f32, kind="ExternalInput").ap()
    gcnb_d = nc.dram_tensor("gcn_b", [KD, 128], f32, kind="ExternalInput").ap()
    fhw1_d = nc.dram_tensor("fh_w1", [KD, 128, H1], f32, kind="ExternalInput").ap()
    fhb1_d = nc.dram_tensor("fh_b1", [KH1, 128], f32, kind="ExternalInput").ap()
    fhw2_d = nc.dram_tensor("fh_w2", [KH1, 128, H2], f32, kind="ExternalInput").ap()
    fhb2_d = nc.dram_tensor("fh_b2", [KH2, 128], f32, kind="ExternalInput").ap()
    fhwh_d = nc.dram_tensor("fh_wh", [KH2, 128, 4], f32, kind="ExternalInput").ap()
    fhbh_d = nc.dram_tensor("fh_bh", [4], f32, kind="ExternalInput").ap()

    OUT_d = nc.dram_tensor("OUT", [8, P], f32, kind="ExternalOutput").ap()

    pid = nc.partition_id()
    groups = [list(range(NCORE))]

    with tile.TileContext(nc) as tc, ExitStack() as top:
        const = top.enter_context(tc.tile_pool(name="const", bufs=1))
        dram = top.enter_context(tc.tile_pool(name="dram", bufs=1, space="DRAM"))

        # V0T spill grouped [m = s%8][d = s//8][128][P]: consecutive-s write
        # batches are one 3D AP, strided-octet reads are one 3D AP.
        V0T_t = dram.tile([8, NIT, 128, P], f32)
        RSEND_t = dram.tile([NIT, NCORE, 128, P], u8)
        RRECV_t = dram.tile([NIT, NCORE, 128, P], u8)
        TMY_t = dram.tile([NIT, 128], f32)
        GD_t = dram.tile([1, P], f32)
        GALL_t = dram.tile([NCORE, P], f32)

        # ---- constants
        iota_i = const.tile([128, 128], dt.int32)
        nc.gpsimd.iota(iota_i[:], pattern=[[1, 128]], base=0, channel_multiplier=0)
        pidx_i = const.tile([128, 1], dt.int32)
        nc.gpsimd.iota(pidx_i[:], pattern=[[0, 1]], base=0, channel_multiplier=1)
        iota_f = const.tile([128, 128], f32)
        nc.vector.tensor_copy(iota_f[:], iota_i[:])
        pidx_f = const.tile([128, 1], f32)
        nc.vector.tensor_copy(pidx_f[:], pidx_i[:])
        eye = const.tile([128, 128], f32)
        nc.vector.tensor_scalar(eye[:], iota_f[:], pidx_f[:, 0:1], None, ALU.is_equal)
        ident = const.tile([128, 128], f32)
        nc.vector.tensor_copy(ident[:], eye[:])
        ones1 = const.tile([1, 128], f32)
        nc.vector.memset(ones1[:], 1.0)
        ones_f = const.tile([128, 1], f32)
        nc.vector.memset(ones_f[:], 1.0)
        ones_r = const.tile([128, 1], f32r)
        nc.vector.tensor_copy(ones_r[:], ones_f[:])
        addv = const.tile([128, 1], f32)
        nc.vector.tensor_scalar(addv[:], pidx_f[:], 2.0, None, ALU.is_equal)
        nc.vector.tensor_scalar(addv[:], addv[:], 1.0, 1e-6, ALU.mult, ALU.add)

        def load_kmaj(pool, dram_ap, kt, cols, dtype=f32, tag=None):
            t = pool.tile([128, kt, cols], dtype, tag=tag or f"w_{dram_ap.tensor.name}")
            nc.sync.dma_start(out=t[:], in_=dram_ap[:, :, :].bitcast(dtype)
                              .rearrange("k p c -> p k c"))
            return t

        def load_bias(pool, dram_ap, kt):
            tg = f"b_{dram_ap.tensor.name}"
            if kt == 0:
                t = pool.tile([4, 1], f32, tag=tg)
                nc.sync.dma_start(out=t[:, 0:1], in_=dram_ap[0:4])
            else:
                t = pool.tile([128, kt], f32, tag=tg)
                nc.sync.dma_start(out=t[:], in_=dram_ap[:, :].rearrange("k p -> p k"))
            return t

        t2rep = const.tile([128, P], f32)

        # ================= early phase: XWT, head1, G =================
        xw_stack = ExitStack()
        xwP = xw_stack.enter_context(tc.tile_pool(name="xwP", bufs=1))
        xwhi = xwP.tile([128, KD, P], bf16, tag="xwhi")
        xwlo = xwP.tile([128, KD, P], bf16, tag="xwlo")
        with tc.tile_pool(name="early", bufs=1) as early, \
             tc.tile_pool(name="psE", bufs=1, space="PSUM") as psE:
            xmyt = early.tile([128, KD, P], f32)
            nc.sync.dma_start(out=xmyt[:],
                              in_=XMYT_d[:, :, :].rearrange("k p c -> p k c"))
            Wsb = load_kmaj(early, W_d, KD, D)
            for m in range(KD):
                ps = psE.tile([128, P], f32, tag="pxw")
                for h in range(2):
                    for k in range(KD):
                        nc.tensor.matmul(ps[:, h * 512:(h + 1) * 512],
                                         Wsb[:, k, m * 128:(m + 1) * 128],
                                         xmyt[:, k, h * 512:(h + 1) * 512],
                                         start=(k == 0), stop=(k == KD - 1))
                nc.scalar.activation(xwhi[:, m, :], ps[:], AF.Copy)
                nc.vector.tensor_sub(xwlo[:, m, :], ps[:], xwhi[:, m, :])

            ihw1 = load_kmaj(early, ihw1_d, KD, H1, dt.float32r)
            ihw2 = load_kmaj(early, ihw2_d, KH1, H2, dt.float32r)
            ihwh = load_kmaj(early, ihwh_d, KH2, 4, dt.float32r)
            xmyt_r = early.tile([128, KD, P], dt.float32r, tag="xmyt_r")
            nc.sync.dma_start(out=xmyt_r[:],
                              in_=XMYT_d[:, :, :].bitcast(dt.float32r)
                              .rearrange("k p c -> p k c"))
            ihb1 = load_bias(early, ihb1_d, KH1)
            ihb2 = load_bias(early, ihb2_d, KH2)
            ihbh = load_bias(early, ihbh_d, 0)
            with tc.tile_pool(name="hpool", bufs=1) as hpool, \
                 tc.tile_pool(name="psE2", bufs=2, space="PSUM") as psE2:
                u0 = _head(nc, tc, psE2, ihw1, ihb1, ihw2, ihb2, ihwh, ihbh,
                           xmyt_r, OUT_d, 0, True, hpool, addv)
                sg = hpool.tile([1, P], f32, tag="sg")
                _sigmoid(nc, hpool, sg[:], u0[:], [1, P])
                gmy = hpool.tile([1, P], f32, tag="gmy")
                nc.vector.tensor_scalar(gmy[:], sg[:], float(np.float32(-gam)),
                                        1.0, ALU.mult, ALU.add)
                nc.sync.dma_start(out=GD_t[0:1, :], in_=gmy[0:1, :])
                nc.gpsimd.collective_compute("AllGather", ALU.bypass,
                                             replica_groups=groups,
                                             ins=[GD_t.opt()], outs=[GALL_t.opt()])

        # ================= phase A =================
        NPAIR_RUN = NPAIR if KPHASE != 0 else 1
        with tc.tile_pool(name="stripeP", bufs=3) as stripeP, \
             tc.tile_pool(name="pa", bufs=2) as pa, \
             tc.tile_pool(name="pam", bufs=2) as pam, \
             tc.tile_pool(name="pam1", bufs=1) as pam1, \
             tc.tile_pool(name="psA", bufs=2, space="PSUM") as psA, \
             tc.tile_pool(name="psT", bufs=4, space="PSUM") as psT:
            for pr in range(NPAIR_RUN):
                stripes = []
                for i01 in range(2):
                    st = stripeP.tile([128, N], f32, tag="v0")
                    stripes.append(st)
                for jc in range(NJC):
                    xh = pa.tile([128, KD, JC], bf16, tag="xth")
                    xl = pa.tile([128, KD, JC], bf16, tag="xtl")
                    nc.sync.dma_start(
                        out=xh[:], in_=XTHI_d[:, :, jc * JC:(jc + 1) * JC]
                        .rearrange("k p c -> p k c"))
                    nc.sync.dma_start(
                        out=xl[:], in_=XTLO_d[:, :, jc * JC:(jc + 1) * JC]
                        .rearrange("k p c -> p k c"))
                    if jc == 0:
                        # A-row loads issue after the first X^T chunk so the
                        # 8MB transfer does not delay the first matmuls
                        for i01 in range(2):
                            it = pr * 2 + i01
                            nc.sync.dma_start(
                                out=stripes[i01][:],
                                in_=AROW_d[it * 128:(it + 1) * 128, :])
                    for i01 in range(2):
                        it = pr * 2 + i01
                        ps = psA.tile([128, JC], f32, tag=f"psv{i01}")
                        first = True
                        for pi, (aa, bb) in enumerate(
                                ((xwhi, xh), (xwhi, xl), (xwlo, xh))):
                            for k in range(KD):
                                nc.tensor.matmul(
                                    ps[:], aa[:, k, it * 128:(it + 1) * 128],
                                    bb[:, k, :],
                                    start=first, stop=(pi == 2 and k == KD - 1))
                                first = False
                        rel = pa.tile([128, JC], f32, tag=f"rel{i01}")
                        nc.scalar.activation(rel[:], ps[:], AF.Relu,
                                             scale=float(np.float32(1.0 / beta)))
                        sl = stripes[i01][:, jc * JC:(jc + 1) * JC]
                        nc.gpsimd.tensor_add(sl, sl, rel[:])
                for i01 in range(2):
                    it = pr * 2 + i01
                    stripe = stripes[i01]
                    top8 = pam.tile([128, 8], f32, tag="top8")
                    nc.vector.max(top8[:], stripe[:])
                    nc.sync.dma_start(out=TMY_t[it:it + 1, :], in_=top8[:, 4:5])
                    off = nc.snap(pid * P + it * 128, min_val=0, max_val=N - 128)
                    dsub = stripe[:, bass.ds(off, 128)]
                    nc.vector.scalar_tensor_tensor(dsub, eye[:], -1e9, dsub,
                                                   ALU.mult, ALU.add)
                    rmask = pam1.tile([128, N], u8, tag="rmask")
                    nc.vector.tensor_scalar(rmask[:], stripe[:], top8[:, 4:5], None,
                                            ALU.is_ge)
                    nc.sync.dma_start(
                        out=RSEND_t[it].rearrange("c p j -> p c j"), in_=rmask[:])
                    for d8 in range(NIT):
                        ct = pa.tile([128, 8, 128], f32, tag="ctr")
                        for m8 in range(8):
                            s = d8 * 8 + m8
                            pst = psT.tile([128, 128], f32, tag="ptr")
                            nc.tensor.transpose(pst[:], stripe[:, s * 128:(s + 1) * 128],
                                                ident[:])
                            nc.scalar.activation(ct[:, m8, :], pst[:], AF.Copy)
                        nc.sync.dma_start(
                            out=V0T_t[:, d8, :, it * 128:(it + 1) * 128]
                            .rearrange("m p c -> p m c"),
                            in_=ct[:])
                    nc.gpsimd.collective_compute(
                        "AllToAll", ALU.bypass, replica_groups=groups,
                        ins=[RSEND_t[it].opt()], outs=[RRECV_t[it].opt()])

        # T2rep broadcast (exact fp32 K=1 matmul)
        trow = const.tile([1, P], f32)
        nc.sync.dma_start(out=trow[0:1, :], in_=TMY_t[:])
        if KPHASE >= 2:
          with tc.tile_pool(name="psB1", bufs=1, space="PSUM") as psB1:
            for h in range(2):
                psb = psB1.tile([128, 512], f32, tag="pbc")
                nc.tensor.matmul(psb[:], ones1[:], trow[0:1, h * 512:(h + 1) * 512],
                                 start=True, stop=True)
                nc.scalar.activation(t2rep[:, h * 512:(h + 1) * 512], psb[:], AF.Copy)

        # ================= phase B =================
        xw_stack.close()
        if KPHASE >= 2:
            bc = top.enter_context(tc.tile_pool(name="bc", bufs=1))
            pt_acc = bc.tile([128, KD, P], f32, tag="pt_acc")
            rs_acc = bc.tile([1, P], f32, tag="rs_acc")
            gcnw = load_kmaj(bc, gcnw_d, KD, D, f32r)
            gcnb = load_bias(bc, gcnb_d, KD)
            with tc.tile_pool(name="pb", bufs=1) as pb, \
                 tc.tile_pool(name="pbm", bufs=2) as pbm, \
                 tc.tile_pool(name="pbt", bufs=1) as pbt, \
                 tc.tile_pool(name="agtP", bufs=1) as agtP, \
                 tc.tile_pool(name="psP", bufs=1, space="PSUM") as psP, \
                 tc.tile_pool(name="psR", bufs=1, space="PSUM") as psR:
                for o in range(8):
                    # strided octet: j-tiles jt = o + 8*l for l = 0..7.
                    # bufs=1 pools with per-half tags: octet o+1's first-half
                    # load overlaps octet o's second-half compute.
                    v0t, xt_, rcv = [], [], []
                    for half in range(2):
                        vt = pb.tile([128, 4, P], f32, tag=f"v0t{half}")
                        nc.sync.dma_start(
                            out=vt[:],
                            in_=V0T_t[o, half * 4:(half + 1) * 4]
                            .rearrange("d p c -> p d c"))
                        v0t.append(vt)
                        rc = pb.tile([128, 4, P], u8, tag=f"rcv{half}")
                        nc.sync.dma_start(
                            out=rc[:],
                            in_=RRECV_t[o, half * 4:(half + 1) * 4]
                            .rearrange("c p i -> p c i"))
                        rcv.append(rc)
                        xt = pb.tile([128, 4, D], f32, tag=f"xrow{half}")
                        nc.sync.dma_start(
                            out=xt[:],
                            in_=X_d[half * 4:(half + 1) * 4, o]
                            .rearrange("a p d -> p a d"))
                        xt_.append(xt)
                    gsl = pbm.tile([128, NCORE], f32, tag="gsl")
                    nc.sync.dma_start(
                        out=gsl[:],
                        in_=GALL_t[:, o * 128:(o + 1) * 128].rearrange("l p -> p l"))
                    agts, xgs = [], []
                    for l in range(8):
                        vt = v0t[l // 4][:, l % 4, :]
                        mlt = pbm.tile([128, P], u8, tag="mlt")
                        nc.vector.tensor_tensor(mlt[:], vt, t2rep[:], ALU.is_ge)
                        msk = pbm.tile([128, P], u8, tag="msk")
                        nc.vector.tensor_tensor(msk[:], mlt[:],
                                                rcv[l // 4][:, l % 4, :], ALU.max)
                        agt = agtP.tile([128, P], f32r, tag=f"agt{l}")
                        nc.vector.tensor_tensor(agt[:], vt, msk[:], ALU.mult)
                        agts.append(agt)
                        xg = agtP.tile([128, D], f32r, tag=f"xg{l}")
                        nc.scalar.activation(xg[:], xt_[l // 4][:, l % 4, :],
                                             AF.Copy, scale=gsl[:, l:l + 1])
                        xgs.append(xg)
                    for h in range(2):
                        pp = psP.tile([128, KD, 512], f32, tag="pp")
                        for l in range(8):
                            for m in range(KD):
                                nc.tensor.matmul(pp[:, m, :],
                                                 xgs[l][:, m * 128:(m + 1) * 128],
                                                 agts[l][:, h * 512:(h + 1) * 512],
                                                 start=(l == 0), stop=(l == 7))
                        # drain PSUM via ACT, accumulate on Pool: keeps DVE
                        # free for the mask/agt chain
                        if o == 0:
                            for m in range(KD):
                                nc.scalar.activation(
                                    pt_acc[:, m, h * 512:(h + 1) * 512],
                                    pp[:, m, :], AF.Copy)
                        else:
                            ptmp = pbt.tile([128, KD, 512], f32, tag="ptmp")
                            for m in range(KD):
                                nc.scalar.activation(ptmp[:, m, :], pp[:, m, :],
                                                     AF.Copy)
                            for m in range(KD):
                                nc.gpsimd.tensor_add(
                                    pt_acc[:, m, h * 512:(h + 1) * 512],
                                    pt_acc[:, m, h * 512:(h + 1) * 512],
                                    ptmp[:, m, :])
                    for h in range(2):
                        pr2 = psR.tile([1, 512], f32, tag="pr")
                        for l in range(8):
                            nc.tensor.matmul(pr2[0:1, :],
                                             ones_r[:, 0:1],
                                             agts[l][:, h * 512:(h + 1) * 512],
                                             start=(l == 0), stop=(l == 7))
                        if o == 0:
                            nc.vector.tensor_copy(rs_acc[0:1, h * 512:(h + 1) * 512],
                                                  pr2[:])
                        else:
                            nc.vector.tensor_add(rs_acc[0:1, h * 512:(h + 1) * 512],
                                                 rs_acc[0:1, h * 512:(h + 1) * 512],
                                                 pr2[:])

        # ================= phase C =================
        if KPHASE >= 3:
            with tc.tile_pool(name="pc", bufs=1) as pc, \
                 tc.tile_pool(name="hpool2", bufs=1) as hpool2, \
                 tc.tile_pool(name="psC", bufs=1, space="PSUM") as psC, \
                 tc.tile_pool(name="psCh", bufs=2, space="PSUM") as psCh:
                fhw1 = load_kmaj(pc, fhw1_d, KD, H1, f32r)
                fhw2 = load_kmaj(pc, fhw2_d, KH1, H2, f32r)
                fhwh = load_kmaj(pc, fhwh_d, KH2, 4, f32r)
                fhb1 = load_bias(pc, fhb1_d, KH1)
                fhb2 = load_bias(pc, fhb2_d, KH2)
                fhbh = load_bias(pc, fhbh_d, 0)
                pt_acc_r = pc.tile([128, KD, P], f32r, tag="pt_acc_r")
                nc.vector.tensor_copy(pt_acc_r[:], pt_acc[:])
                dinv = pc.tile([1, P], f32, tag="dinv")
                nc.vector.tensor_scalar(dinv[:], rs_acc[:], float(np.float32(eps2)),
                                        None, ALU.max)
                nc.vector.reciprocal(dinv[:], dinv[:])
                drep = pc.tile([128, P], f32)
                psb = psC.tile([128, P], f32, tag="pxw")
                for h in range(2):
                    nc.tensor.matmul(psb[:, h * 512:(h + 1) * 512], ones1[:],
                                     dinv[0:1, h * 512:(h + 1) * 512],
                                     start=True, stop=True)
                nc.scalar.activation(drep[:], psb[:], AF.Copy)

                xmyt = pc.tile([128, KD, P], f32, tag="xmyt2")
                nc.sync.dma_start(out=xmyt[:],
                                  in_=XMYT_d[:, :, :].rearrange("k p c -> p k c"))

                xpm = pc.tile([128, KD, P], f32r)
                for m in range(KD):
                    ps = psC.tile([128, P], f32, tag="pxw")
                    for h in range(2):
                        for k in range(KD):
                            nc.tensor.matmul(ps[:, h * 512:(h + 1) * 512],
                                             gcnw[:, k, m * 128:(m + 1) * 128],
                                             pt_acc_r[:, k, h * 512:(h + 1) * 512],
                                             start=(k == 0), stop=(k == KD - 1))
                    tmp = pc.tile([128, P], f32, tag="mtmp")
                    nc.vector.tensor_mul(tmp[:], ps[:], drep[:])
                    mf = pc.tile([128, P], f32, tag="mf")
                    nc.scalar.activation(mf[:], tmp[:], AF.Gelu, bias=gcnb[:, m:m + 1])
                    nc.vector.tensor_add(xpm[:, m, :], xmyt[:, m, :], mf[:])

                _head(nc, tc, psCh, fhw1, fhb1, fhw2, fhb2, fhwh, fhbh,
                      xpm, OUT_d, 4, False, hpool2, addv)

    nc.finalize()
    return nc


_NC_CACHE = {}
_last_in_maps = None


def kernel(**inputs) -> tuple:
    X = np.ascontiguousarray(np.asarray(inputs["X"], dtype=np.float32))
    A = np.asarray(inputs["A"], dtype=np.float32)
    ra = float(np.asarray(inputs["ra"], dtype=np.float64))
    gam = float(np.asarray(inputs["gam"], dtype=np.float64))
    al = float(np.float32(1.0) / (np.float32(1.0) + np.float32(np.exp(-np.float32(ra)))))
    beta = al / (1.0 - al)
    eps2 = 1e-8 / al

    XT = np.ascontiguousarray(X.T)
    XTHI = XT.astype(ml_dtypes.bfloat16)
    XTLO = (XT - XTHI.astype(np.float32)).astype(ml_dtypes.bfloat16)

    key = (round(beta, 12), round(gam, 12), KPHASE)
    if key not in _NC_CACHE:
        _NC_CACHE[key] = build_nc(beta, gam, eps2)
    nc = _NC_CACHE[key]

    rep = {
        "X": X.reshape(NIT, NCORE, 128, D),
        "XTHI": XTHI.reshape(KD, 128, N),
        "XTLO": XTLO.reshape(KD, 128, N),
        "W_gm": None, "gcn_w": None,
    }
    for k, kt, cols in (("W_gm", KD, D), ("ih_w1", KD, H1), ("ih_w2", KH1, H2),
                        ("ih_wh", KH2, 4), ("gcn_w", KD, D), ("fh_w1", KD, H1),
                        ("fh_w2", KH1, H2), ("fh_wh", KH2, 4)):
        rep[k] = np.ascontiguousarray(
            np.asarray(inputs[k], dtype=np.float32)).reshape(kt, 128, cols)
    for k, kt in (("ih_b1", KH1), ("ih_b2", KH2), ("gcn_b", KD),
                  ("fh_b1", KH1), ("fh_b2", KH2)):
        rep[k] = np.ascontiguousarray(
            np.asarray(inputs[k], dtype=np.float32)).reshape(kt, 128)
    for k in ("ih_bh", "fh_bh"):
        rep[k] = np.ascontiguousarray(np.asarray(inputs[k], dtype=np.float32))

    in_maps = []
    for c in range(NCORE):
        m = dict(rep)
        m["XMYT"] = np.ascontiguousarray(XT[:, c * P:(c + 1) * P]).reshape(KD, 128, P)
        m["AROW"] = np.ascontiguousarray(A[c * P:(c + 1) * P, :])
        in_maps.append(m)

    global _last_in_maps
    _last_in_maps = in_maps
    res = run_bass_kernel_spmd(nc, in_maps, list(range(NCORE)))
    full = np.concatenate([res.results[c]["OUT"] for c in range(NCORE)], axis=1)
    return tuple(full[i] for i in range(8))


if __name__ == "__main__":
    import jax
    import reference
    cpu = jax.devices("cpu")[0]
    with jax.default_device(cpu):
        inp = reference.setup_inputs()
        inp = {k: np.asarray(v) for k, v in inp.items()}
    got = kernel(**inp)
    with jax.default_device(cpu):
        exp = [np.asarray(x) for x in reference.reference(
            **{k: jax.device_put(v, cpu) for k, v in inp.items()})]
    for i, (g, e) in enumerate(zip(got, exp)):
        e = np.asarray(e)
        err = np.abs(g - e).max()
        rel = err / max(np.abs(e).max(), 1e-9)
        print(f"out{i}: maxabs {err:.3e} rel {rel:.3e}")


# revision 4
# speedup vs baseline: 1.1722x; 1.0024x over previous
"""Trainium2 Bass kernel for nn_EvidentialGSL (8-core row-sharded), v2.

Same algorithm as the baseline kernel (see kernel.py docstring) with the
hot-loop restructured around DMA-issue cost and SBUF reuse:
  - every multi-tile load/store is a single 2/3-dim-AP DMA (HWDGE fixed
    overhead is ~625ns per dma_start, independent of size)
  - phase A processes i-tiles in pairs so the X^T hi/lo stream is read
    4x instead of 8x; A rows are DMA'd straight into the stripe and the
    relu(S) term is accumulated in place
  - V0T spill tiles are grouped [m=s%8][d=s//8] so phase B can fetch a
    strided octet (all j-tiles = o mod 8) with one DMA and start as soon
    as AllToAll #o lands
  - phase B element-wise work is split across DVE (compare/mult) and
    GpSimd (mask max)
"""
import os
import numpy as np
from contextlib import ExitStack

KPHASE = int(os.environ.get("KPHASE", "3"))

import ml_dtypes
from concourse import bass, bacc, tile, mybir
from concourse.bass_utils import run_bass_kernel_spmd

dt = mybir.dt
AF = mybir.ActivationFunctionType
ALU = mybir.AluOpType

N, D = 8192, 768
H1, H2 = 512, 256
NCORE = 8
P = N // NCORE          # 1024 rows per core
NIT = P // 128          # 8 i-tiles per core
NJT = N // 128          # 64 j-tiles
KD = D // 128           # 6
KH1 = H1 // 128         # 4
KH2 = H2 // 128         # 2
JC = 512                # phase-A j chunk
NJC = N // JC           # 16
NPAIR = NIT // 2        # 4 passes of i-tile pairs


def _softplus(nc, pool, out_ap, in_ap, shp, neg=False):
    t1 = pool.tile(shp, dt.float32, tag="sp_a")
    t2 = pool.tile(shp, dt.float32, tag="sp_b")
    nc.scalar.activation(t1[:], in_ap, AF.Abs)
    nc.scalar.activation(t1[:], t1[:], AF.Exp, scale=-1.0)
    nc.scalar.activation(t1[:], t1[:], AF.Ln, bias=1.0)
    nc.scalar.activation(t2[:], in_ap, AF.Relu, scale=(-1.0 if neg else 1.0))
    nc.vector.tensor_add(out_ap, t1[:], t2[:])


def _sigmoid(nc, pool, out_ap, in_ap, shp):
    t3 = pool.tile(shp, dt.float32, tag="sp_c")
    _softplus(nc, pool, t3[:], in_ap, shp, neg=True)
    nc.scalar.activation(out_ap, t3[:], AF.Exp, scale=-1.0)


def _head(nc, tc, psum, w1sb, b1sb, w2sb, b2sb, whsb, bhsb, xin, out_dram,
          obase, want_u0, hpool, addv):
    """Transposed NIG head on xin [128, KD, P] float32r; writes 4 output rows.

    Head matmuls run in f32r (1 cyc/row vs fp32's 4): ~1.6e-4 relative on the
    head outputs, well inside the 2e-2 budget and with no top-k sensitivity.
    """
    h1 = hpool.tile([128, KH1, P], dt.float32r, tag="h1t")
    for m in range(KH1):
        ps = psum.tile([128, P], dt.float32, tag="ph")
        for h in range(2):
            for k in range(KD):
                nc.tensor.matmul(ps[:, h * 512:(h + 1) * 512],
                                 w1sb[:, k, m * 128:(m + 1) * 128],
                                 xin[:, k, h * 512:(h + 1) * 512],
                                 start=(k == 0), stop=(k == KD - 1))
        nc.scalar.activation(h1[:, m, :], ps[:], AF.Gelu, bias=b1sb[:, m:m + 1])
    h2 = hpool.tile([128, KH2, P], dt.float32r, tag="h2t")
    for m in range(KH2):
        ps = psum.tile([128, P], dt.float32, tag="ph")
        for h in range(2):
            for k in range(KH1):
                nc.tensor.matmul(ps[:, h * 512:(h + 1) * 512],
                                 w2sb[:, k, m * 128:(m + 1) * 128],
                                 h1[:, k, h * 512:(h + 1) * 512],
                                 start=(k == 0), stop=(k == KH1 - 1))
        nc.scalar.activation(h2[:, m, :], ps[:], AF.Gelu, bias=b2sb[:, m:m + 1])
    ps4 = psum.tile([4, P], dt.float32, tag="ph")
    for h in range(2):
        for k in range(KH2):
            nc.tensor.matmul(ps4[:, h * 512:(h + 1) * 512], whsb[:, k, 0:4],
                             h2[:, k, h * 512:(h + 1) * 512],
                             start=(k == 0), stop=(k == KH2 - 1))
    r4 = hpool.tile([4, P], dt.float32, tag="r4")
    nc.scalar.activation(r4[:], ps4[:], AF.Identity, bias=bhsb[0:4, 0:1])
    nc.sync.dma_start(out=out_dram[obase:obase + 1, :], in_=r4[0:1, :])
    o1 = hpool.tile([4, P], dt.float32, tag="o4")
    _softplus(nc, hpool, o1[:], r4[:], [4, P])
    nc.vector.tensor_scalar(o1[:], o1[:], addv[0:4, 0:1], None, ALU.add)
    nc.sync.dma_start(out=out_dram[obase + 1:obase + 4, :], in_=o1[1:4, :])
    if not want_u0:
        return None
    a0t = hpool.tile([1, P], dt.float32, tag="a0t")
    b0t = hpool.tile([1, P], dt.float32, tag="b0t")
    nc.sync.dma_start(out=a0t[:], in_=o1[2:3, :])
    nc.sync.dma_start(out=b0t[:], in_=o1[3:4, :])
    nc.vector.tensor_scalar(a0t[:], a0t[:], -1.0, 1e-8, ALU.add, ALU.max)
    nc.vector.reciprocal(a0t[:], a0t[:])
    u0 = hpool.tile([1, P], dt.float32, tag="u0")
    nc.vector.tensor_mul(u0[:], b0t[:], a0t[:])
    return u0


def _head_h(nc, psum, w1sb, b1sb, w2sb, b2sb, whsb, bhsb, xin, out_dram,
            obase, hpool, addv, hh):
    """Half-width (512-col) variant of _head for one column half hh."""
    c0, c1 = hh * 512, (hh + 1) * 512
    h1 = hpool.tile([128, KH1, 512], dt.float32r, tag="h1h")
    for m in range(KH1):
        ps = psum.tile([128, 512], dt.float32, tag="ph")
        for k in range(KD):
            nc.tensor.matmul(ps[:], w1sb[:, k, m * 128:(m + 1) * 128],
                             xin[:, k, c0:c1], start=(k == 0), stop=(k == KD - 1))
        nc.scalar.activation(h1[:, m, :], ps[:], AF.Gelu, bias=b1sb[:, m:m + 1])
    h2 = hpool.tile([128, KH2, 512], dt.float32r, tag="h2h")
    for m in range(KH2):
        ps = psum.tile([128, 512], dt.float32, tag="ph")
        for k in range(KH1):
            nc.tensor.matmul(ps[:], w2sb[:, k, m * 128:(m + 1) * 128],
                             h1[:, k, :], start=(k == 0), stop=(k == KH1 - 1))
        nc.scalar.activation(h2[:, m, :], ps[:], AF.Gelu, bias=b2sb[:, m:m + 1])
    ps4 = psum.tile([4, 512], dt.float32, tag="ph")
    for k in range(KH2):
        nc.tensor.matmul(ps4[:], whsb[:, k, 0:4], h2[:, k, :],
                         start=(k == 0), stop=(k == KH2 - 1))
    r4 = hpool.tile([4, 512], dt.float32, tag="r4h")
    nc.scalar.activation(r4[:], ps4[:], AF.Identity, bias=bhsb[0:4, 0:1])
    nc.sync.dma_start(out=out_dram[obase:obase + 1, c0:c1], in_=r4[0:1, :])
    o1 = hpool.tile([4, 512], dt.float32, tag="o4h")
    _softplus(nc, hpool, o1[:], r4[:], [4, 512])
    nc.vector.tensor_scalar(o1[:], o1[:], addv[0:4, 0:1], None, ALU.add)
    nc.sync.dma_start(out=out_dram[obase + 1:obase + 4, c0:c1], in_=o1[1:4, :])



def build_nc(beta: float, gam: float, eps2: float):
    nc = bacc.Bacc("TRN2", target_bir_lowering=False, debug=False,
                   num_devices=NCORE)
    f32, f32r, bf16, u8 = dt.float32, dt.float32r, dt.bfloat16, dt.uint8

    # X reshaped [a][b][128][D] with j-tile jt = a*8 + b, so a strided octet
    # (fixed b) is a single 3D-AP DMA.
    X_d = nc.dram_tensor("X", [NIT, NCORE, 128, D], f32, kind="ExternalInput").ap()
    XTHI_d = nc.dram_tensor("XTHI", [KD, 128, N], bf16, kind="ExternalInput").ap()
    XTLO_d = nc.dram_tensor("XTLO", [KD, 128, N], bf16, kind="ExternalInput").ap()
    XMYT_d = nc.dram_tensor("XMYT", [KD, 128, P], f32, kind="ExternalInput").ap()
    AROW_d = nc.dram_tensor("AROW", [P, N], f32, kind="ExternalInput").ap()
    W_d = nc.dram_tensor("W_gm", [KD, 128, D], f32, kind="ExternalInput").ap()
    ihw1_d = nc.dram_tensor("ih_w1", [KD, 128, H1], f32, kind="ExternalInput").ap()
    ihb1_d = nc.dram_tensor("ih_b1", [KH1, 128], f32, kind="ExternalInput").ap()
    ihw2_d = nc.dram_tensor("ih_w2", [KH1, 128, H2], f32, kind="ExternalInput").ap()
    ihb2_d = nc.dram_tensor("ih_b2", [KH2, 128], f32, kind="ExternalInput").ap()
    ihwh_d = nc.dram_tensor("ih_wh", [KH2, 128, 4], f32, kind="ExternalInput").ap()
    ihbh_d = nc.dram_tensor("ih_bh", [4], f32, kind="ExternalInput").ap()
    gcnw_d = nc.dram_tensor("gcn_w", [KD, 128, D], f32, kind="ExternalInput").ap()
    gcnb_d = nc.dram_tensor("gcn_b", [KD, 128], f32, kind="ExternalInput").ap()
    fhw1_d = nc.dram_tensor("fh_w1", [KD, 128, H1], f32, kind="ExternalInput").ap()
    fhb1_d = nc.dram_tensor("fh_b1", [KH1, 128], f32, kind="ExternalInput").ap()
    fhw2_d = nc.dram_tensor("fh_w2", [KH1, 128, H2], f32, kind="ExternalInput").ap()
    fhb2_d = nc.dram_tensor("fh_b2", [KH2, 128], f32, kind="ExternalInput").ap()
    fhwh_d = nc.dram_tensor("fh_wh", [KH2, 128, 4], f32, kind="ExternalInput").ap()
    fhbh_d = nc.dram_tensor("fh_bh", [4], f32, kind="ExternalInput").ap()

    OUT_d = nc.dram_tensor("OUT", [8, P], f32, kind="ExternalOutput").ap()

    pid = nc.partition_id()
    groups = [list(range(NCORE))]

    with tile.TileContext(nc) as tc, ExitStack() as top:
        const = top.enter_context(tc.tile_pool(name="const", bufs=1))
        dram = top.enter_context(tc.tile_pool(name="dram", bufs=1, space="DRAM"))

        # V0T spill grouped [m = s%8][d = s//8][128][P]: consecutive-s write
        # batches are one 3D AP, strided-octet reads are one 3D AP.
        V0T_t = dram.tile([8, NIT, 128, P], f32)
        RSEND_t = dram.tile([NIT, NCORE, 128, P], u8)
        RRECV_t = dram.tile([NIT, NCORE, 128, P], u8)
        TMY_t = dram.tile([NIT, 128], f32)
        GD_t = dram.tile([1, P], f32)
        GALL_t = dram.tile([NCORE, P], f32)

        # ---- constants
        iota_i = const.tile([128, 128], dt.int32)
        nc.gpsimd.iota(iota_i[:], pattern=[[1, 128]], base=0, channel_multiplier=0)
        pidx_i = const.tile([128, 1], dt.int32)
        nc.gpsimd.iota(pidx_i[:], pattern=[[0, 1]], base=0, channel_multiplier=1)
        iota_f = const.tile([128, 128], f32)
        nc.vector.tensor_copy(iota_f[:], iota_i[:])
        pidx_f = const.tile([128, 1], f32)
        nc.vector.tensor_copy(pidx_f[:], pidx_i[:])
        eye = const.tile([128, 128], f32)
        nc.vector.tensor_scalar(eye[:], iota_f[:], pidx_f[:, 0:1], None, ALU.is_equal)
        ident = const.tile([128, 128], f32)
        nc.vector.tensor_copy(ident[:], eye[:])
        ones1 = const.tile([1, 128], f32)
        nc.vector.memset(ones1[:], 1.0)
        ones_f = const.tile([128, 1], f32)
        nc.vector.memset(ones_f[:], 1.0)
        ones_r = const.tile([128, 1], f32r)
        nc.vector.tensor_copy(ones_r[:], ones_f[:])
        addv = const.tile([128, 1], f32)
        nc.vector.tensor_scalar(addv[:], pidx_f[:], 2.0, None, ALU.is_equal)
        nc.vector.tensor_scalar(addv[:], addv[:], 1.0, 1e-6, ALU.mult, ALU.add)

        def load_kmaj(pool, dram_ap, kt, cols, dtype=f32, tag=None):
            t = pool.tile([128, kt, cols], dtype, tag=tag or f"w_{dram_ap.tensor.name}")
            nc.sync.dma_start(out=t[:], in_=dram_ap[:, :, :].bitcast(dtype)
                              .rearrange("k p c -> p k c"))
            return t

        def load_bias(pool, dram_ap, kt):
            tg = f"b_{dram_ap.tensor.name}"
            if kt == 0:
                t = pool.tile([4, 1], f32, tag=tg)
                nc.sync.dma_start(out=t[:, 0:1], in_=dram_ap[0:4])
            else:
                t = pool.tile([128, kt], f32, tag=tg)
                nc.sync.dma_start(out=t[:], in_=dram_ap[:, :].rearrange("k p -> p k"))
            return t

        t2rep = const.tile([128, P], f32)

        # ================= early phase: XWT, head1, G =================
        xw_stack = ExitStack()
        xwP = xw_stack.enter_context(tc.tile_pool(name="xwP", bufs=1))
        xwhi = xwP.tile([128, KD, P], bf16, tag="xwhi")
        xwlo = xwP.tile([128, KD, P], bf16, tag="xwlo")
        with tc.tile_pool(name="early", bufs=1) as early, \
             tc.tile_pool(name="psE", bufs=1, space="PSUM") as psE:
            xmyt = early.tile([128, KD, P], f32)
            nc.sync.dma_start(out=xmyt[:],
                              in_=XMYT_d[:, :, :].rearrange("k p c -> p k c"))
            Wsb = load_kmaj(early, W_d, KD, D)
            for m in range(KD):
                ps = psE.tile([128, P], f32, tag="pxw")
                for h in range(2):
                    for k in range(KD):
                        nc.tensor.matmul(ps[:, h * 512:(h + 1) * 512],
                                         Wsb[:, k, m * 128:(m + 1) * 128],
                                         xmyt[:, k, h * 512:(h + 1) * 512],
                                         start=(k == 0), stop=(k == KD - 1))
                nc.scalar.activation(xwhi[:, m, :], ps[:], AF.Copy)
                nc.vector.tensor_sub(xwlo[:, m, :], ps[:], xwhi[:, m, :])

            ihw1 = load_kmaj(early, ihw1_d, KD, H1, dt.float32r)
            ihw2 = load_kmaj(early, ihw2_d, KH1, H2, dt.float32r)
            ihwh = load_kmaj(early, ihwh_d, KH2, 4, dt.float32r)
            xmyt_r = early.tile([128, KD, P], dt.float32r, tag="xmyt_r")
            nc.sync.dma_start(out=xmyt_r[:],
                              in_=XMYT_d[:, :, :].bitcast(dt.float32r)
                              .rearrange("k p c -> p k c"))
            ihb1 = load_bias(early, ihb1_d, KH1)
            ihb2 = load_bias(early, ihb2_d, KH2)
            ihbh = load_bias(early, ihbh_d, 0)
            with tc.tile_pool(name="hpool", bufs=1) as hpool, \
                 tc.tile_pool(name="psE2", bufs=2, space="PSUM") as psE2:
                u0 = _head(nc, tc, psE2, ihw1, ihb1, ihw2, ihb2, ihwh, ihbh,
                           xmyt_r, OUT_d, 0, True, hpool, addv)
                sg = hpool.tile([1, P], f32, tag="sg")
                _sigmoid(nc, hpool, sg[:], u0[:], [1, P])
                gmy = hpool.tile([1, P], f32, tag="gmy")
                nc.vector.tensor_scalar(gmy[:], sg[:], float(np.float32(-gam)),
                                        1.0, ALU.mult, ALU.add)
                nc.sync.dma_start(out=GD_t[0:1, :], in_=gmy[0:1, :])
                nc.gpsimd.collective_compute("AllGather", ALU.bypass,
                                             replica_groups=groups,
                                             ins=[GD_t.opt()], outs=[GALL_t.opt()])

        # ================= phase A =================
        NPAIR_RUN = NPAIR if KPHASE != 0 else 1
        with tc.tile_pool(name="stripeP", bufs=3) as stripeP, \
             tc.tile_pool(name="pa", bufs=2) as pa, \
             tc.tile_pool(name="pam", bufs=2) as pam, \
             tc.tile_pool(name="pam1", bufs=1) as pam1, \
             tc.tile_pool(name="psA", bufs=2, space="PSUM") as psA, \
             tc.tile_pool(name="psT", bufs=4, space="PSUM") as psT:
            for pr in range(NPAIR_RUN):
                stripes = []
                for i01 in range(2):
                    st = stripeP.tile([128, N], f32, tag="v0")
                    stripes.append(st)
                for jc in range(NJC):
                    xh = pa.tile([128, KD, JC], bf16, tag="xth")
                    xl = pa.tile([128, KD, JC], bf16, tag="xtl")
                    nc.sync.dma_start(
                        out=xh[:], in_=XTHI_d[:, :, jc * JC:(jc + 1) * JC]
                        .rearrange("k p c -> p k c"))
                    nc.sync.dma_start(
                        out=xl[:], in_=XTLO_d[:, :, jc * JC:(jc + 1) * JC]
                        .rearrange("k p c -> p k c"))
                    if jc == 0:
                        # A-row loads issue after the first X^T chunk so the
                        # 8MB transfer does not delay the first matmuls
                        for i01 in range(2):
                            it = pr * 2 + i01
                            nc.sync.dma_start(
                                out=stripes[i01][:],
                                in_=AROW_d[it * 128:(it + 1) * 128, :])
                    for i01 in range(2):
                        it = pr * 2 + i01
                        ps = psA.tile([128, JC], f32, tag=f"psv{i01}")
                        first = True
                        for pi, (aa, bb) in enumerate(
                                ((xwhi, xh), (xwhi, xl), (xwlo, xh))):
                            for k in range(KD):
                                nc.tensor.matmul(
                                    ps[:], aa[:, k, it * 128:(it + 1) * 128],
                                    bb[:, k, :],
                                    start=first, stop=(pi == 2 and k == KD - 1))
                                first = False
                        rel = pa.tile([128, JC], f32, tag=f"rel{i01}")
                        nc.scalar.activation(rel[:], ps[:], AF.Relu,
                                             scale=float(np.float32(1.0 / beta)))
                        sl = stripes[i01][:, jc * JC:(jc + 1) * JC]
                        nc.gpsimd.tensor_add(sl, sl, rel[:])
                for i01 in range(2):
                    it = pr * 2 + i01
                    stripe = stripes[i01]
                    top8 = pam.tile([128, 8], f32, tag="top8")
                    nc.vector.max(top8[:], stripe[:])
                    nc.sync.dma_start(out=TMY_t[it:it + 1, :], in_=top8[:, 4:5])
                    off = nc.snap(pid * P + it * 128, min_val=0, max_val=N - 128)
                    dsub = stripe[:, bass.ds(off, 128)]
                    nc.vector.scalar_tensor_tensor(dsub, eye[:], -1e9, dsub,
                                                   ALU.mult, ALU.add)
                    rmask = pam1.tile([128, N], u8, tag="rmask")
                    nc.vector.tensor_scalar(rmask[:], stripe[:], top8[:, 4:5], None,
                                            ALU.is_ge)
                    nc.sync.dma_start(
                        out=RSEND_t[it].rearrange("c p j -> p c j"), in_=rmask[:])
                    for d8 in range(NIT):
                        ct = pa.tile([128, 8, 128], f32, tag="ctr")
                        for m8 in range(8):
                            s = d8 * 8 + m8
                            pst = psT.tile([128, 128], f32, tag="ptr")
                            nc.tensor.transpose(pst[:], stripe[:, s * 128:(s + 1) * 128],
                                                ident[:])
                            nc.scalar.activation(ct[:, m8, :], pst[:], AF.Copy)
                        nc.sync.dma_start(
                            out=V0T_t[:, d8, :, it * 128:(it + 1) * 128]
                            .rearrange("m p c -> p m c"),
                            in_=ct[:])
                    nc.gpsimd.collective_compute(
                        "AllToAll", ALU.bypass, replica_groups=groups,
                        ins=[RSEND_t[it].opt()], outs=[RRECV_t[it].opt()])


        # T2rep broadcast (exact fp32 K=1 matmul)
        trow = const.tile([1, P], f32)
        nc.sync.dma_start(out=trow[0:1, :], in_=TMY_t[:])
        if KPHASE >= 2:
          with tc.tile_pool(name="psB1", bufs=1, space="PSUM") as psB1:
            for h in range(2):
                psb = psB1.tile([128, 512], f32, tag="pbc")
                nc.tensor.matmul(psb[:], ones1[:], trow[0:1, h * 512:(h + 1) * 512],
                                 start=True, stop=True)
                nc.scalar.activation(t2rep[:, h * 512:(h + 1) * 512], psb[:], AF.Copy)

        # ================= phase B =================
        xw_stack.close()
        if KPHASE >= 2:
            bc = top.enter_context(tc.tile_pool(name="bc", bufs=1))
            pt_acc = bc.tile([128, KD, P], f32, tag="pt_acc")
            rs_acc = bc.tile([1, P], f32, tag="rs_acc")
            gcnw = load_kmaj(bc, gcnw_d, KD, D, f32r)
            gcnb = load_bias(bc, gcnb_d, KD)
            with tc.tile_pool(name="pb", bufs=1) as pb, \
                 tc.tile_pool(name="pbm", bufs=2) as pbm, \
                 tc.tile_pool(name="pbt", bufs=1) as pbt, \
                 tc.tile_pool(name="agtP", bufs=1) as agtP, \
                 tc.tile_pool(name="psP", bufs=1, space="PSUM") as psP, \
                 tc.tile_pool(name="psR", bufs=1, space="PSUM") as psR:
                for o in range(8):
                    # strided octet: j-tiles jt = o + 8*l for l = 0..7.
                    # bufs=1 pools with per-half tags: octet o+1's first-half
                    # load overlaps octet o's second-half compute.
                    v0t, xt_, rcv = [], [], []
                    for half in range(2):
                        vt = pb.tile([128, 4, P], f32, tag=f"v0t{half}")
                        nc.sync.dma_start(
                            out=vt[:],
                            in_=V0T_t[o, half * 4:(half + 1) * 4]
                            .rearrange("d p c -> p d c"))
                        v0t.append(vt)
                        rc = pb.tile([128, 4, P], u8, tag=f"rcv{half}")
                        nc.sync.dma_start(
                            out=rc[:],
                            in_=RRECV_t[o, half * 4:(half + 1) * 4]
                            .rearrange("c p i -> p c i"))
                        rcv.append(rc)
                        xt = pb.tile([128, 4, D], f32, tag=f"xrow{half}")
                        nc.sync.dma_start(
                            out=xt[:],
                            in_=X_d[half * 4:(half + 1) * 4, o]
                            .rearrange("a p d -> p a d"))
                        xt_.append(xt)
                    gsl = pbm.tile([128, NCORE], f32, tag="gsl")
                    nc.sync.dma_start(
                        out=gsl[:],
                        in_=GALL_t[:, o * 128:(o + 1) * 128].rearrange("l p -> p l"))
                    agts, xgs = [], []
                    for l in range(8):
                        vt = v0t[l // 4][:, l % 4, :]
                        mlt = pbm.tile([128, P], u8, tag="mlt")
                        nc.vector.tensor_tensor(mlt[:], vt, t2rep[:], ALU.is_ge)
                        msk = pbm.tile([128, P], u8, tag="msk")
                        nc.vector.tensor_tensor(msk[:], mlt[:],
                                                rcv[l // 4][:, l % 4, :], ALU.max)
                        agt = agtP.tile([128, P], f32r, tag=f"agt{l}")
                        nc.vector.tensor_tensor(agt[:], vt, msk[:], ALU.mult)
                        agts.append(agt)
                        xg = agtP.tile([128, D], f32r, tag=f"xg{l}")
                        nc.scalar.activation(xg[:], xt_[l // 4][:, l % 4, :],
                                             AF.Copy, scale=gsl[:, l:l + 1])
                        xgs.append(xg)
                    for h in range(2):
                        pp = psP.tile([128, KD, 512], f32, tag="pp")
                        for l in range(8):
                            for m in range(KD):
                                nc.tensor.matmul(pp[:, m, :],
                                                 xgs[l][:, m * 128:(m + 1) * 128],
                                                 agts[l][:, h * 512:(h + 1) * 512],
                                                 start=(l == 0), stop=(l == 7))
                        # drain PSUM via ACT, accumulate on Pool: keeps DVE
                        # free for the mask/agt chain
                        if o == 0:
                            for m in range(KD):
                                nc.scalar.activation(
                                    pt_acc[:, m, h * 512:(h + 1) * 512],
                                    pp[:, m, :], AF.Copy)
                        else:
                            ptmp = pbt.tile([128, KD, 512], f32, tag="ptmp")
                            for m in range(KD):
                                nc.scalar.activation(ptmp[:, m, :], pp[:, m, :],
                                                     AF.Copy)
                            for m in range(KD):
                                nc.gpsimd.tensor_add(
                                    pt_acc[:, m, h * 512:(h + 1) * 512],
                                    pt_acc[:, m, h * 512:(h + 1) * 512],
                                    ptmp[:, m, :])
                    for h in range(2):
                        pr2 = psR.tile([1, 512], f32, tag="pr")
                        for l in range(8):
                            nc.tensor.matmul(pr2[0:1, :],
                                             ones_r[:, 0:1],
                                             agts[l][:, h * 512:(h + 1) * 512],
                                             start=(l == 0), stop=(l == 7))
                        if o == 0:
                            nc.vector.tensor_copy(rs_acc[0:1, h * 512:(h + 1) * 512],
                                                  pr2[:])
                        else:
                            nc.vector.tensor_add(rs_acc[0:1, h * 512:(h + 1) * 512],
                                                 rs_acc[0:1, h * 512:(h + 1) * 512],
                                                 pr2[:])

        # ================= phase C =================
        if KPHASE >= 3:
            with tc.tile_pool(name="pc", bufs=1) as pc, \
                 tc.tile_pool(name="hpool2", bufs=1) as hpool2, \
                 tc.tile_pool(name="psC", bufs=1, space="PSUM") as psC, \
                 tc.tile_pool(name="psCh", bufs=2, space="PSUM") as psCh:
                fhw1 = load_kmaj(pc, fhw1_d, KD, H1, f32r)
                fhw2 = load_kmaj(pc, fhw2_d, KH1, H2, f32r)
                fhwh = load_kmaj(pc, fhwh_d, KH2, 4, f32r)
                fhb1 = load_bias(pc, fhb1_d, KH1)
                fhb2 = load_bias(pc, fhb2_d, KH2)
                fhbh = load_bias(pc, fhbh_d, 0)
                pt_acc_r = pc.tile([128, KD, P], f32r, tag="pt_acc_r")
                nc.vector.tensor_copy(pt_acc_r[:], pt_acc[:])
                dinv = pc.tile([1, P], f32, tag="dinv")
                nc.vector.tensor_scalar(dinv[:], rs_acc[:], float(np.float32(eps2)),
                                        None, ALU.max)
                nc.vector.reciprocal(dinv[:], dinv[:])
                drep = pc.tile([128, P], f32)
                psb = psC.tile([128, P], f32, tag="pbc")
                for h in range(2):
                    nc.tensor.matmul(psb[:, h * 512:(h + 1) * 512], ones1[:],
                                     dinv[0:1, h * 512:(h + 1) * 512],
                                     start=True, stop=True)
                nc.scalar.activation(drep[:], psb[:], AF.Copy)

                xmyt = pc.tile([128, KD, P], f32, tag="xmyt2")
                nc.sync.dma_start(out=xmyt[:],
                                  in_=XMYT_d[:, :, :].rearrange("k p c -> p k c"))

                # column-half pipeline: the fh head on half 0 overlaps the
                # gcn/gelu production of half 1
                xpm = pc.tile([128, KD, P], f32r)
                for hh in range(2):
                    c0, c1 = hh * 512, (hh + 1) * 512
                    for m in range(KD):
                        ps = psC.tile([128, 512], f32, tag="pxw")
                        for k in range(KD):
                            nc.tensor.matmul(ps[:],
                                             gcnw[:, k, m * 128:(m + 1) * 128],
                                             pt_acc_r[:, k, c0:c1],
                                             start=(k == 0), stop=(k == KD - 1))
                        tmp = pc.tile([128, 512], f32, tag="mtmp")
                        nc.vector.tensor_mul(tmp[:], ps[:], drep[:, c0:c1])
                        mf = pc.tile([128, 512], f32, tag="mf")
                        nc.scalar.activation(mf[:], tmp[:], AF.Gelu,
                                             bias=gcnb[:, m:m + 1])
                        nc.vector.tensor_add(xpm[:, m, c0:c1], xmyt[:, m, c0:c1],
                                             mf[:])
                    _head_h(nc, psCh, fhw1, fhb1, fhw2, fhb2, fhwh, fhbh,
                            xpm, OUT_d, 4, hpool2, addv, hh)

    nc.finalize()
    return nc


_NC_CACHE = {}
_last_in_maps = None


def kernel(**inputs) -> tuple:
    X = np.ascontiguousarray(np.asarray(inputs["X"], dtype=np.float32))
    A = np.asarray(inputs["A"], dtype=np.float32)
    ra = float(np.asarray(inputs["ra"], dtype=np.float64))
    gam = float(np.asarray(inputs["gam"], dtype=np.float64))
    al = float(np.float32(1.0) / (np.float32(1.0) + np.float32(np.exp(-np.float32(ra)))))
    beta = al / (1.0 - al)
    eps2 = 1e-8 / al

    XT = np.ascontiguousarray(X.T)
    XTHI = XT.astype(ml_dtypes.bfloat16)
    XTLO = (XT - XTHI.astype(np.float32)).astype(ml_dtypes.bfloat16)

    key = (round(beta, 12), round(gam, 12), KPHASE)
    if key not in _NC_CACHE:
        _NC_CACHE[key] = build_nc(beta, gam, eps2)
    nc = _NC_CACHE[key]

    rep = {
        "X": X.reshape(NIT, NCORE, 128, D),
        "XTHI": XTHI.reshape(KD, 128, N),
        "XTLO": XTLO.reshape(KD, 128, N),
        "W_gm": None, "gcn_w": None,
    }
    for k, kt, cols in (("W_gm", KD, D), ("ih_w1", KD, H1), ("ih_w2", KH1, H2),
                        ("ih_wh", KH2, 4), ("gcn_w", KD, D), ("fh_w1", KD, H1),
                        ("fh_w2", KH1, H2), ("fh_wh", KH2, 4)):
        rep[k] = np.ascontiguousarray(
            np.asarray(inputs[k], dtype=np.float32)).reshape(kt, 128, cols)
    for k, kt in (("ih_b1", KH1), ("ih_b2", KH2), ("gcn_b", KD),
                  ("fh_b1", KH1), ("fh_b2", KH2)):
        rep[k] = np.ascontiguousarray(
            np.asarray(inputs[k], dtype=np.float32)).reshape(kt, 128)
    for k in ("ih_bh", "fh_bh"):
        rep[k] = np.ascontiguousarray(np.asarray(inputs[k], dtype=np.float32))

    in_maps = []
    for c in range(NCORE):
        m = dict(rep)
        m["XMYT"] = np.ascontiguousarray(XT[:, c * P:(c + 1) * P]).reshape(KD, 128, P)
        m["AROW"] = np.ascontiguousarray(A[c * P:(c + 1) * P, :])
        in_maps.append(m)

    global _last_in_maps
    _last_in_maps = in_maps
    res = run_bass_kernel_spmd(nc, in_maps, list(range(NCORE)))
    full = np.concatenate([res.results[c]["OUT"] for c in range(NCORE)], axis=1)
    return tuple(full[i] for i in range(8))


if __name__ == "__main__":
    import jax
    import reference
    cpu = jax.devices("cpu")[0]
    with jax.default_device(cpu):
        inp = reference.setup_inputs()
        inp = {k: np.asarray(v) for k, v in inp.items()}
    got = kernel(**inp)
    with jax.default_device(cpu):
        exp = [np.asarray(x) for x in reference.reference(
            **{k: jax.device_put(v, cpu) for k, v in inp.items()})]
    for i, (g, e) in enumerate(zip(got, exp)):
        e = np.asarray(e)
        err = np.abs(g - e).max()
        rel = err / max(np.abs(e).max(), 1e-9)
        print(f"out{i}: maxabs {err:.3e} rel {rel:.3e}")


# revision 5
# speedup vs baseline: 1.2223x; 1.0427x over previous
"""Trainium2 Bass kernel for nn_EvidentialGSL (8-core row-sharded), v2.

Same algorithm as the baseline kernel (see kernel.py docstring) with the
hot-loop restructured around DMA-issue cost and SBUF reuse:
  - every multi-tile load/store is a single 2/3-dim-AP DMA (HWDGE fixed
    overhead is ~625ns per dma_start, independent of size)
  - phase A processes i-tiles in pairs so the X^T hi/lo stream is read
    4x instead of 8x; A rows are DMA'd straight into the stripe and the
    relu(S) term is accumulated in place
  - V0T spill tiles are grouped [m=s%8][d=s//8] so phase B can fetch a
    strided octet (all j-tiles = o mod 8) with one DMA and start as soon
    as AllToAll #o lands
  - phase B element-wise work is split across DVE (compare/mult) and
    GpSimd (mask max)
"""
import os
import numpy as np
from contextlib import ExitStack

KPHASE = int(os.environ.get("KPHASE", "3"))

import ml_dtypes
from concourse import bass, bacc, tile, mybir
from concourse.bass_utils import run_bass_kernel_spmd

dt = mybir.dt
AF = mybir.ActivationFunctionType
ALU = mybir.AluOpType

N, D = 8192, 768
H1, H2 = 512, 256
NCORE = 8
P = N // NCORE          # 1024 rows per core
NIT = P // 128          # 8 i-tiles per core
NJT = N // 128          # 64 j-tiles
KD = D // 128           # 6
KH1 = H1 // 128         # 4
KH2 = H2 // 128         # 2
JC = 512                # phase-A j chunk
NJC = N // JC           # 16
NPAIR = NIT // 2        # 4 passes of i-tile pairs


def _softplus(nc, pool, out_ap, in_ap, shp, neg=False):
    t1 = pool.tile(shp, dt.float32, tag="sp_a")
    t2 = pool.tile(shp, dt.float32, tag="sp_b")
    nc.scalar.activation(t1[:], in_ap, AF.Abs)
    nc.scalar.activation(t1[:], t1[:], AF.Exp, scale=-1.0)
    nc.scalar.activation(t1[:], t1[:], AF.Ln, bias=1.0)
    nc.scalar.activation(t2[:], in_ap, AF.Relu, scale=(-1.0 if neg else 1.0))
    nc.vector.tensor_add(out_ap, t1[:], t2[:])


def _sigmoid(nc, pool, out_ap, in_ap, shp):
    t3 = pool.tile(shp, dt.float32, tag="sp_c")
    _softplus(nc, pool, t3[:], in_ap, shp, neg=True)
    nc.scalar.activation(out_ap, t3[:], AF.Exp, scale=-1.0)


def _head(nc, tc, psum, w1sb, b1sb, w2sb, b2sb, whsb, bhsb, xin, out_dram,
          obase, want_u0, hpool, addv):
    """Transposed NIG head on xin [128, KD, P] float32r; writes 4 output rows.

    Head matmuls run in f32r (1 cyc/row vs fp32's 4): ~1.6e-4 relative on the
    head outputs, well inside the 2e-2 budget and with no top-k sensitivity.
    """
    h1 = hpool.tile([128, KH1, P], dt.float32r, tag="h1t")
    for m in range(KH1):
        ps = psum.tile([128, P], dt.float32, tag="ph")
        for h in range(2):
            for k in range(KD):
                nc.tensor.matmul(ps[:, h * 512:(h + 1) * 512],
                                 w1sb[:, k, m * 128:(m + 1) * 128],
                                 xin[:, k, h * 512:(h + 1) * 512],
                                 start=(k == 0), stop=(k == KD - 1))
        nc.scalar.activation(h1[:, m, :], ps[:], AF.Gelu, bias=b1sb[:, m:m + 1])
    h2 = hpool.tile([128, KH2, P], dt.float32r, tag="h2t")
    for m in range(KH2):
        ps = psum.tile([128, P], dt.float32, tag="ph")
        for h in range(2):
            for k in range(KH1):
                nc.tensor.matmul(ps[:, h * 512:(h + 1) * 512],
                                 w2sb[:, k, m * 128:(m + 1) * 128],
                                 h1[:, k, h * 512:(h + 1) * 512],
                                 start=(k == 0), stop=(k == KH1 - 1))
        nc.scalar.activation(h2[:, m, :], ps[:], AF.Gelu, bias=b2sb[:, m:m + 1])
    ps4 = psum.tile([4, P], dt.float32, tag="ph")
    for h in range(2):
        for k in range(KH2):
            nc.tensor.matmul(ps4[:, h * 512:(h + 1) * 512], whsb[:, k, 0:4],
                             h2[:, k, h * 512:(h + 1) * 512],
                             start=(k == 0), stop=(k == KH2 - 1))
    r4 = hpool.tile([4, P], dt.float32, tag="r4")
    nc.scalar.activation(r4[:], ps4[:], AF.Identity, bias=bhsb[0:4, 0:1])
    nc.sync.dma_start(out=out_dram[obase:obase + 1, :], in_=r4[0:1, :])
    o1 = hpool.tile([4, P], dt.float32, tag="o4")
    _softplus(nc, hpool, o1[:], r4[:], [4, P])
    nc.vector.tensor_scalar(o1[:], o1[:], addv[0:4, 0:1], None, ALU.add)
    nc.sync.dma_start(out=out_dram[obase + 1:obase + 4, :], in_=o1[1:4, :])
    if not want_u0:
        return None
    a0t = hpool.tile([1, P], dt.float32, tag="a0t")
    b0t = hpool.tile([1, P], dt.float32, tag="b0t")
    nc.sync.dma_start(out=a0t[:], in_=o1[2:3, :])
    nc.sync.dma_start(out=b0t[:], in_=o1[3:4, :])
    nc.vector.tensor_scalar(a0t[:], a0t[:], -1.0, 1e-8, ALU.add, ALU.max)
    nc.vector.reciprocal(a0t[:], a0t[:])
    u0 = hpool.tile([1, P], dt.float32, tag="u0")
    nc.vector.tensor_mul(u0[:], b0t[:], a0t[:])
    return u0


def _head_h(nc, psum, w1sb, b1sb, w2sb, b2sb, whsb, bhsb, xin, out_dram,
            obase, hpool, addv, hh):
    """Half-width (512-col) variant of _head for one column half hh."""
    c0, c1 = hh * 512, (hh + 1) * 512
    h1 = hpool.tile([128, KH1, 512], dt.float32r, tag="h1h")
    for m in range(KH1):
        ps = psum.tile([128, 512], dt.float32, tag="ph")
        for k in range(KD):
            nc.tensor.matmul(ps[:], w1sb[:, k, m * 128:(m + 1) * 128],
                             xin[:, k, c0:c1], start=(k == 0), stop=(k == KD - 1))
        nc.scalar.activation(h1[:, m, :], ps[:], AF.Gelu, bias=b1sb[:, m:m + 1])
    h2 = hpool.tile([128, KH2, 512], dt.float32r, tag="h2h")
    for m in range(KH2):
        ps = psum.tile([128, 512], dt.float32, tag="ph")
        for k in range(KH1):
            nc.tensor.matmul(ps[:], w2sb[:, k, m * 128:(m + 1) * 128],
                             h1[:, k, :], start=(k == 0), stop=(k == KH1 - 1))
        nc.scalar.activation(h2[:, m, :], ps[:], AF.Gelu, bias=b2sb[:, m:m + 1])
    ps4 = psum.tile([4, 512], dt.float32, tag="ph")
    for k in range(KH2):
        nc.tensor.matmul(ps4[:], whsb[:, k, 0:4], h2[:, k, :],
                         start=(k == 0), stop=(k == KH2 - 1))
    r4 = hpool.tile([4, 512], dt.float32, tag="r4h")
    nc.scalar.activation(r4[:], ps4[:], AF.Identity, bias=bhsb[0:4, 0:1])
    nc.sync.dma_start(out=out_dram[obase:obase + 1, c0:c1], in_=r4[0:1, :])
    o1 = hpool.tile([4, 512], dt.float32, tag="o4h")
    _softplus(nc, hpool, o1[:], r4[:], [4, 512])
    nc.vector.tensor_scalar(o1[:], o1[:], addv[0:4, 0:1], None, ALU.add)
    nc.sync.dma_start(out=out_dram[obase + 1:obase + 4, c0:c1], in_=o1[1:4, :])



def build_nc(beta: float, gam: float, eps2: float):
    nc = bacc.Bacc("TRN2", target_bir_lowering=False, debug=False,
                   num_devices=NCORE)
    f32, f32r, bf16, u8 = dt.float32, dt.float32r, dt.bfloat16, dt.uint8

    # X reshaped [a][b][128][D] with j-tile jt = a*8 + b, so a strided octet
    # (fixed b) is a single 3D-AP DMA.
    X_d = nc.dram_tensor("X", [NIT, NCORE, 128, D], f32, kind="ExternalInput").ap()
    XTHI_d = nc.dram_tensor("XTHI", [KD, 128, N], bf16, kind="ExternalInput").ap()
    XTLO_d = nc.dram_tensor("XTLO", [KD, 128, N], bf16, kind="ExternalInput").ap()
    XMYT_d = nc.dram_tensor("XMYT", [KD, 128, P], f32, kind="ExternalInput").ap()
    AROW_d = nc.dram_tensor("AROW", [P, N], f32, kind="ExternalInput").ap()
    W_d = nc.dram_tensor("W_gm", [KD, 128, D], f32, kind="ExternalInput").ap()
    ihw1_d = nc.dram_tensor("ih_w1", [KD, 128, H1], f32, kind="ExternalInput").ap()
    ihb1_d = nc.dram_tensor("ih_b1", [KH1, 128], f32, kind="ExternalInput").ap()
    ihw2_d = nc.dram_tensor("ih_w2", [KH1, 128, H2], f32, kind="ExternalInput").ap()
    ihb2_d = nc.dram_tensor("ih_b2", [KH2, 128], f32, kind="ExternalInput").ap()
    ihwh_d = nc.dram_tensor("ih_wh", [KH2, 128, 4], f32, kind="ExternalInput").ap()
    ihbh_d = nc.dram_tensor("ih_bh", [4], f32, kind="ExternalInput").ap()
    gcnw_d = nc.dram_tensor("gcn_w", [KD, 128, D], f32, kind="ExternalInput").ap()
    gcnb_d = nc.dram_tensor("gcn_b", [KD, 128], f32, kind="ExternalInput").ap()
    fhw1_d = nc.dram_tensor("fh_w1", [KD, 128, H1], f32, kind="ExternalInput").ap()
    fhb1_d = nc.dram_tensor("fh_b1", [KH1, 128], f32, kind="ExternalInput").ap()
    fhw2_d = nc.dram_tensor("fh_w2", [KH1, 128, H2], f32, kind="ExternalInput").ap()
    fhb2_d = nc.dram_tensor("fh_b2", [KH2, 128], f32, kind="ExternalInput").ap()
    fhwh_d = nc.dram_tensor("fh_wh", [KH2, 128, 4], f32, kind="ExternalInput").ap()
    fhbh_d = nc.dram_tensor("fh_bh", [4], f32, kind="ExternalInput").ap()

    OUT_d = nc.dram_tensor("OUT", [8, P], f32, kind="ExternalOutput").ap()

    pid = nc.partition_id()
    groups = [list(range(NCORE))]

    with tile.TileContext(nc) as tc, ExitStack() as top:
        const = top.enter_context(tc.tile_pool(name="const", bufs=1))
        dram = top.enter_context(tc.tile_pool(name="dram", bufs=1, space="DRAM"))

        # V0T spill grouped [m = s%8][d = s//8][128][P]: consecutive-s write
        # batches are one 3D AP, strided-octet reads are one 3D AP.
        V0T_t = dram.tile([8, NIT, 128, P], f32)
        RSEND_t = dram.tile([NIT, NCORE, 128, P], u8)
        RRECV_t = dram.tile([NIT, NCORE, 128, P], u8)
        TMY_t = dram.tile([NIT, 128], f32)
        GD_t = dram.tile([1, P], f32)
        GALL_t = dram.tile([NCORE, P], f32)

        # ---- constants
        iota_i = const.tile([128, 128], dt.int32)
        nc.gpsimd.iota(iota_i[:], pattern=[[1, 128]], base=0, channel_multiplier=0)
        pidx_i = const.tile([128, 1], dt.int32)
        nc.gpsimd.iota(pidx_i[:], pattern=[[0, 1]], base=0, channel_multiplier=1)
        iota_f = const.tile([128, 128], f32)
        nc.vector.tensor_copy(iota_f[:], iota_i[:])
        pidx_f = const.tile([128, 1], f32)
        nc.vector.tensor_copy(pidx_f[:], pidx_i[:])
        eye = const.tile([128, 128], f32)
        nc.vector.tensor_scalar(eye[:], iota_f[:], pidx_f[:, 0:1], None, ALU.is_equal)
        ident = const.tile([128, 128], f32)
        nc.vector.tensor_copy(ident[:], eye[:])
        ones1 = const.tile([1, 128], f32)
        nc.vector.memset(ones1[:], 1.0)
        ones_f = const.tile([128, 1], f32)
        nc.vector.memset(ones_f[:], 1.0)
        ones_r = const.tile([128, 1], f32r)
        nc.vector.tensor_copy(ones_r[:], ones_f[:])
        addv = const.tile([128, 1], f32)
        nc.vector.tensor_scalar(addv[:], pidx_f[:], 2.0, None, ALU.is_equal)
        nc.vector.tensor_scalar(addv[:], addv[:], 1.0, 1e-6, ALU.mult, ALU.add)

        def load_kmaj(pool, dram_ap, kt, cols, dtype=f32, tag=None):
            t = pool.tile([128, kt, cols], dtype, tag=tag or f"w_{dram_ap.tensor.name}")
            nc.sync.dma_start(out=t[:], in_=dram_ap[:, :, :].bitcast(dtype)
                              .rearrange("k p c -> p k c"))
            return t

        def load_bias(pool, dram_ap, kt):
            tg = f"b_{dram_ap.tensor.name}"
            if kt == 0:
                t = pool.tile([4, 1], f32, tag=tg)
                nc.sync.dma_start(out=t[:, 0:1], in_=dram_ap[0:4])
            else:
                t = pool.tile([128, kt], f32, tag=tg)
                nc.sync.dma_start(out=t[:], in_=dram_ap[:, :].rearrange("k p -> p k"))
            return t

        t2rep = const.tile([128, P], f32)

        # ================= early phase: XWT, head1, G =================
        xw_stack = ExitStack()
        xwP = xw_stack.enter_context(tc.tile_pool(name="xwP", bufs=1))
        xwhi = xwP.tile([128, KD, P], bf16, tag="xwhi")
        xwlo = xwP.tile([128, KD, P], bf16, tag="xwlo")
        with tc.tile_pool(name="early", bufs=1) as early, \
             tc.tile_pool(name="psE", bufs=1, space="PSUM") as psE:
            xmyt = early.tile([128, KD, P], f32)
            nc.sync.dma_start(out=xmyt[:],
                              in_=XMYT_d[:, :, :].rearrange("k p c -> p k c"))
            Wsb = load_kmaj(early, W_d, KD, D)
            for m in range(KD):
                ps = psE.tile([128, P], f32, tag="pxw")
                for h in range(2):
                    for k in range(KD):
                        nc.tensor.matmul(ps[:, h * 512:(h + 1) * 512],
                                         Wsb[:, k, m * 128:(m + 1) * 128],
                                         xmyt[:, k, h * 512:(h + 1) * 512],
                                         start=(k == 0), stop=(k == KD - 1))
                nc.scalar.activation(xwhi[:, m, :], ps[:], AF.Copy)
                nc.vector.tensor_sub(xwlo[:, m, :], ps[:], xwhi[:, m, :])

            ihw1 = load_kmaj(early, ihw1_d, KD, H1, dt.float32r)
            ihw2 = load_kmaj(early, ihw2_d, KH1, H2, dt.float32r)
            ihwh = load_kmaj(early, ihwh_d, KH2, 4, dt.float32r)
            xmyt_r = early.tile([128, KD, P], dt.float32r, tag="xmyt_r")
            nc.sync.dma_start(out=xmyt_r[:],
                              in_=XMYT_d[:, :, :].bitcast(dt.float32r)
                              .rearrange("k p c -> p k c"))
            ihb1 = load_bias(early, ihb1_d, KH1)
            ihb2 = load_bias(early, ihb2_d, KH2)
            ihbh = load_bias(early, ihbh_d, 0)
            with tc.tile_pool(name="hpool", bufs=1) as hpool, \
                 tc.tile_pool(name="psE2", bufs=2, space="PSUM") as psE2:
                u0 = _head(nc, tc, psE2, ihw1, ihb1, ihw2, ihb2, ihwh, ihbh,
                           xmyt_r, OUT_d, 0, True, hpool, addv)
                sg = hpool.tile([1, P], f32, tag="sg")
                _sigmoid(nc, hpool, sg[:], u0[:], [1, P])
                gmy = hpool.tile([1, P], f32, tag="gmy")
                nc.vector.tensor_scalar(gmy[:], sg[:], float(np.float32(-gam)),
                                        1.0, ALU.mult, ALU.add)
                nc.sync.dma_start(out=GD_t[0:1, :], in_=gmy[0:1, :])
                nc.gpsimd.collective_compute("AllGather", ALU.bypass,
                                             replica_groups=groups,
                                             ins=[GD_t.opt()], outs=[GALL_t.opt()])

        # ================= phase A =================
        NPAIR_RUN = NPAIR if KPHASE != 0 else 1
        with tc.tile_pool(name="stripeP", bufs=3) as stripeP, \
             tc.tile_pool(name="pa", bufs=3) as pa, \
             tc.tile_pool(name="pam", bufs=2) as pam, \
             tc.tile_pool(name="pam1", bufs=1) as pam1, \
             tc.tile_pool(name="psA", bufs=2, space="PSUM") as psA, \
             tc.tile_pool(name="psT", bufs=4, space="PSUM") as psT:
            for pr in range(NPAIR_RUN):
                stripes = []
                for i01 in range(2):
                    st = stripeP.tile([128, N], f32, tag="v0")
                    stripes.append(st)
                for jc in range(NJC):
                    xh = pa.tile([128, KD, JC], bf16, tag="xth")
                    xl = pa.tile([128, KD, JC], bf16, tag="xtl")
                    nc.sync.dma_start(
                        out=xh[:], in_=XTHI_d[:, :, jc * JC:(jc + 1) * JC]
                        .rearrange("k p c -> p k c"))
                    nc.sync.dma_start(
                        out=xl[:], in_=XTLO_d[:, :, jc * JC:(jc + 1) * JC]
                        .rearrange("k p c -> p k c"))
                    if jc == 0:
                        # A-row loads issue after the first X^T chunk so the
                        # 8MB transfer does not delay the first matmuls
                        for i01 in range(2):
                            it = pr * 2 + i01
                            nc.sync.dma_start(
                                out=stripes[i01][:],
                                in_=AROW_d[it * 128:(it + 1) * 128, :])
                    for i01 in range(2):
                        it = pr * 2 + i01
                        ps = psA.tile([128, JC], f32, tag=f"psv{i01}")
                        first = True
                        for pi, (aa, bb) in enumerate(
                                ((xwhi, xh), (xwhi, xl), (xwlo, xh))):
                            for k in range(KD):
                                nc.tensor.matmul(
                                    ps[:], aa[:, k, it * 128:(it + 1) * 128],
                                    bb[:, k, :],
                                    start=first, stop=(pi == 2 and k == KD - 1))
                                first = False
                        rel = pa.tile([128, JC], f32, tag=f"rel{i01}")
                        nc.scalar.activation(rel[:], ps[:], AF.Relu,
                                             scale=float(np.float32(1.0 / beta)))
                        sl = stripes[i01][:, jc * JC:(jc + 1) * JC]
                        nc.gpsimd.tensor_add(sl, sl, rel[:])
                for i01 in range(2):
                    it = pr * 2 + i01
                    stripe = stripes[i01]
                    top8 = pam.tile([128, 8], f32, tag="top8")
                    nc.vector.max(top8[:], stripe[:])
                    nc.sync.dma_start(out=TMY_t[it:it + 1, :], in_=top8[:, 4:5])
                    off = nc.snap(pid * P + it * 128, min_val=0, max_val=N - 128)
                    dsub = stripe[:, bass.ds(off, 128)]
                    nc.vector.scalar_tensor_tensor(dsub, eye[:], -1e9, dsub,
                                                   ALU.mult, ALU.add)
                    rmask = pam1.tile([128, N], u8, tag="rmask")
                    nc.vector.tensor_scalar(rmask[:], stripe[:], top8[:, 4:5], None,
                                            ALU.is_ge)
                    nc.sync.dma_start(
                        out=RSEND_t[it].rearrange("c p j -> p c j"), in_=rmask[:])
                    for d8 in range(NIT):
                        ct = pa.tile([128, 8, 128], f32, tag="ctr")
                        for m8 in range(8):
                            s = d8 * 8 + m8
                            pst = psT.tile([128, 128], f32, tag="ptr")
                            nc.tensor.transpose(pst[:], stripe[:, s * 128:(s + 1) * 128],
                                                ident[:])
                            nc.scalar.activation(ct[:, m8, :], pst[:], AF.Copy)
                        nc.sync.dma_start(
                            out=V0T_t[:, d8, :, it * 128:(it + 1) * 128]
                            .rearrange("m p c -> p m c"),
                            in_=ct[:])
                    nc.gpsimd.collective_compute(
                        "AllToAll", ALU.bypass, replica_groups=groups,
                        ins=[RSEND_t[it].opt()], outs=[RRECV_t[it].opt()])


        # T2rep broadcast (exact fp32 K=1 matmul)
        trow = const.tile([1, P], f32)
        nc.sync.dma_start(out=trow[0:1, :], in_=TMY_t[:])
        if KPHASE >= 2:
          with tc.tile_pool(name="psB1", bufs=1, space="PSUM") as psB1:
            for h in range(2):
                psb = psB1.tile([128, 512], f32, tag="pbc")
                nc.tensor.matmul(psb[:], ones1[:], trow[0:1, h * 512:(h + 1) * 512],
                                 start=True, stop=True)
                nc.scalar.activation(t2rep[:, h * 512:(h + 1) * 512], psb[:], AF.Copy)

        # ================= phase B =================
        xw_stack.close()
        if KPHASE >= 2:
            bc = top.enter_context(tc.tile_pool(name="bc", bufs=1))
            pt_acc = bc.tile([128, KD, P], f32, tag="pt_acc")
            rs_acc = bc.tile([1, P], f32, tag="rs_acc")
            gcnw = load_kmaj(bc, gcnw_d, KD, D, f32r)
            gcnb = load_bias(bc, gcnb_d, KD)
            with tc.tile_pool(name="pb", bufs=1) as pb, \
                 tc.tile_pool(name="pbm", bufs=2) as pbm, \
                 tc.tile_pool(name="pbt", bufs=1) as pbt, \
                 tc.tile_pool(name="agtP", bufs=1) as agtP, \
                 tc.tile_pool(name="psP", bufs=1, space="PSUM") as psP, \
                 tc.tile_pool(name="psR", bufs=1, space="PSUM") as psR:
                for o in range(8):
                    # strided octet: j-tiles jt = o + 8*l for l = 0..7.
                    # bufs=1 pools with per-half tags: octet o+1's first-half
                    # load overlaps octet o's second-half compute.
                    v0t, xt_, rcv = [], [], []
                    for half in range(2):
                        vt = pb.tile([128, 4, P], f32, tag=f"v0t{half}")
                        nc.sync.dma_start(
                            out=vt[:],
                            in_=V0T_t[o, half * 4:(half + 1) * 4]
                            .rearrange("d p c -> p d c"))
                        v0t.append(vt)
                        rc = pb.tile([128, 4, P], u8, tag=f"rcv{half}")
                        nc.sync.dma_start(
                            out=rc[:],
                            in_=RRECV_t[o, half * 4:(half + 1) * 4]
                            .rearrange("c p i -> p c i"))
                        rcv.append(rc)
                        xt = pb.tile([128, 4, D], f32, tag=f"xrow{half}")
                        nc.sync.dma_start(
                            out=xt[:],
                            in_=X_d[half * 4:(half + 1) * 4, o]
                            .rearrange("a p d -> p a d"))
                        xt_.append(xt)
                    gsl = pbm.tile([128, NCORE], f32, tag="gsl")
                    nc.sync.dma_start(
                        out=gsl[:],
                        in_=GALL_t[:, o * 128:(o + 1) * 128].rearrange("l p -> p l"))
                    agts, xgs = [], []
                    for l in range(8):
                        vt = v0t[l // 4][:, l % 4, :]
                        mlt = pbm.tile([128, P], u8, tag="mlt")
                        nc.vector.tensor_tensor(mlt[:], vt, t2rep[:], ALU.is_ge)
                        msk = pbm.tile([128, P], u8, tag="msk")
                        nc.vector.tensor_tensor(msk[:], mlt[:],
                                                rcv[l // 4][:, l % 4, :], ALU.max)
                        agt = agtP.tile([128, P], f32r, tag=f"agt{l}")
                        nc.vector.tensor_tensor(agt[:], vt, msk[:], ALU.mult)
                        agts.append(agt)
                        xg = agtP.tile([128, D], f32r, tag=f"xg{l}")
                        nc.scalar.activation(xg[:], xt_[l // 4][:, l % 4, :],
                                             AF.Copy, scale=gsl[:, l:l + 1])
                        xgs.append(xg)
                    for h in range(2):
                        pp = psP.tile([128, KD, 512], f32, tag="pp")
                        for l in range(8):
                            for m in range(KD):
                                nc.tensor.matmul(pp[:, m, :],
                                                 xgs[l][:, m * 128:(m + 1) * 128],
                                                 agts[l][:, h * 512:(h + 1) * 512],
                                                 start=(l == 0), stop=(l == 7))
                        # drain PSUM via ACT, accumulate on Pool: keeps DVE
                        # free for the mask/agt chain
                        if o == 0:
                            for m in range(KD):
                                nc.scalar.activation(
                                    pt_acc[:, m, h * 512:(h + 1) * 512],
                                    pp[:, m, :], AF.Copy)
                        else:
                            ptmp = pbt.tile([128, KD, 512], f32, tag="ptmp")
                            for m in range(KD):
                                nc.scalar.activation(ptmp[:, m, :], pp[:, m, :],
                                                     AF.Copy)
                            for m in range(KD):
                                nc.gpsimd.tensor_add(
                                    pt_acc[:, m, h * 512:(h + 1) * 512],
                                    pt_acc[:, m, h * 512:(h + 1) * 512],
                                    ptmp[:, m, :])
                    for h in range(2):
                        pr2 = psR.tile([1, 512], f32, tag="pr")
                        for l in range(8):
                            nc.tensor.matmul(pr2[0:1, :],
                                             ones_r[:, 0:1],
                                             agts[l][:, h * 512:(h + 1) * 512],
                                             start=(l == 0), stop=(l == 7))
                        if o == 0:
                            nc.vector.tensor_copy(rs_acc[0:1, h * 512:(h + 1) * 512],
                                                  pr2[:])
                        else:
                            nc.vector.tensor_add(rs_acc[0:1, h * 512:(h + 1) * 512],
                                                 rs_acc[0:1, h * 512:(h + 1) * 512],
                                                 pr2[:])

        # ================= phase C =================
        if KPHASE >= 3:
            with tc.tile_pool(name="pc", bufs=1) as pc, \
                 tc.tile_pool(name="hpool2", bufs=1) as hpool2, \
                 tc.tile_pool(name="psC", bufs=1, space="PSUM") as psC, \
                 tc.tile_pool(name="psCh", bufs=2, space="PSUM") as psCh:
                fhw1 = load_kmaj(pc, fhw1_d, KD, H1, f32r)
                fhw2 = load_kmaj(pc, fhw2_d, KH1, H2, f32r)
                fhwh = load_kmaj(pc, fhwh_d, KH2, 4, f32r)
                fhb1 = load_bias(pc, fhb1_d, KH1)
                fhb2 = load_bias(pc, fhb2_d, KH2)
                fhbh = load_bias(pc, fhbh_d, 0)
                pt_acc_r = pc.tile([128, KD, P], f32r, tag="pt_acc_r")
                nc.vector.tensor_copy(pt_acc_r[:], pt_acc[:])
                dinv = pc.tile([1, P], f32, tag="dinv")
                nc.vector.tensor_scalar(dinv[:], rs_acc[:], float(np.float32(eps2)),
                                        None, ALU.max)
                nc.vector.reciprocal(dinv[:], dinv[:])
                drep = pc.tile([128, P], f32)
                psb = psC.tile([128, P], f32, tag="pbc")
                for h in range(2):
                    nc.tensor.matmul(psb[:, h * 512:(h + 1) * 512], ones1[:],
                                     dinv[0:1, h * 512:(h + 1) * 512],
                                     start=True, stop=True)
                nc.scalar.activation(drep[:], psb[:], AF.Copy)

                xmyt = pc.tile([128, KD, P], f32, tag="xmyt2")
                nc.sync.dma_start(out=xmyt[:],
                                  in_=XMYT_d[:, :, :].rearrange("k p c -> p k c"))

                # column-half pipeline: the fh head on half 0 overlaps the
                # gcn/gelu production of half 1
                xpm = pc.tile([128, KD, P], f32r)
                for hh in range(2):
                    c0, c1 = hh * 512, (hh + 1) * 512
                    for m in range(KD):
                        ps = psC.tile([128, 512], f32, tag="pxw")
                        for k in range(KD):
                            nc.tensor.matmul(ps[:],
                                             gcnw[:, k, m * 128:(m + 1) * 128],
                                             pt_acc_r[:, k, c0:c1],
                                             start=(k == 0), stop=(k == KD - 1))
                        tmp = pc.tile([128, 512], f32, tag="mtmp")
                        nc.vector.tensor_mul(tmp[:], ps[:], drep[:, c0:c1])
                        mf = pc.tile([128, 512], f32, tag="mf")
                        nc.scalar.activation(mf[:], tmp[:], AF.Gelu,
                                             bias=gcnb[:, m:m + 1])
                        nc.vector.tensor_add(xpm[:, m, c0:c1], xmyt[:, m, c0:c1],
                                             mf[:])
                    _head_h(nc, psCh, fhw1, fhb1, fhw2, fhb2, fhwh, fhbh,
                            xpm, OUT_d, 4, hpool2, addv, hh)

    nc.finalize()
    return nc


_NC_CACHE = {}
_last_in_maps = None


def kernel(**inputs) -> tuple:
    X = np.ascontiguousarray(np.asarray(inputs["X"], dtype=np.float32))
    A = np.asarray(inputs["A"], dtype=np.float32)
    ra = float(np.asarray(inputs["ra"], dtype=np.float64))
    gam = float(np.asarray(inputs["gam"], dtype=np.float64))
    al = float(np.float32(1.0) / (np.float32(1.0) + np.float32(np.exp(-np.float32(ra)))))
    beta = al / (1.0 - al)
    eps2 = 1e-8 / al

    XT = np.ascontiguousarray(X.T)
    XTHI = XT.astype(ml_dtypes.bfloat16)
    XTLO = (XT - XTHI.astype(np.float32)).astype(ml_dtypes.bfloat16)

    key = (round(beta, 12), round(gam, 12), KPHASE)
    if key not in _NC_CACHE:
        _NC_CACHE[key] = build_nc(beta, gam, eps2)
    nc = _NC_CACHE[key]

    rep = {
        "X": X.reshape(NIT, NCORE, 128, D),
        "XTHI": XTHI.reshape(KD, 128, N),
        "XTLO": XTLO.reshape(KD, 128, N),
        "W_gm": None, "gcn_w": None,
    }
    for k, kt, cols in (("W_gm", KD, D), ("ih_w1", KD, H1), ("ih_w2", KH1, H2),
                        ("ih_wh", KH2, 4), ("gcn_w", KD, D), ("fh_w1", KD, H1),
                        ("fh_w2", KH1, H2), ("fh_wh", KH2, 4)):
        rep[k] = np.ascontiguousarray(
            np.asarray(inputs[k], dtype=np.float32)).reshape(kt, 128, cols)
    for k, kt in (("ih_b1", KH1), ("ih_b2", KH2), ("gcn_b", KD),
                  ("fh_b1", KH1), ("fh_b2", KH2)):
        rep[k] = np.ascontiguousarray(
            np.asarray(inputs[k], dtype=np.float32)).reshape(kt, 128)
    for k in ("ih_bh", "fh_bh"):
        rep[k] = np.ascontiguousarray(np.asarray(inputs[k], dtype=np.float32))

    in_maps = []
    for c in range(NCORE):
        m = dict(rep)
        m["XMYT"] = np.ascontiguousarray(XT[:, c * P:(c + 1) * P]).reshape(KD, 128, P)
        m["AROW"] = np.ascontiguousarray(A[c * P:(c + 1) * P, :])
        in_maps.append(m)

    global _last_in_maps
    _last_in_maps = in_maps
    res = run_bass_kernel_spmd(nc, in_maps, list(range(NCORE)))
    full = np.concatenate([res.results[c]["OUT"] for c in range(NCORE)], axis=1)
    return tuple(full[i] for i in range(8))


if __name__ == "__main__":
    import jax
    import reference
    cpu = jax.devices("cpu")[0]
    with jax.default_device(cpu):
        inp = reference.setup_inputs()
        inp = {k: np.asarray(v) for k, v in inp.items()}
    got = kernel(**inp)
    with jax.default_device(cpu):
        exp = [np.asarray(x) for x in reference.reference(
            **{k: jax.device_put(v, cpu) for k, v in inp.items()})]
    for i, (g, e) in enumerate(zip(got, exp)):
        e = np.asarray(e)
        err = np.abs(g - e).max()
        rel = err / max(np.abs(e).max(), 1e-9)
        print(f"out{i}: maxabs {err:.3e} rel {rel:.3e}")


# revision 6
# speedup vs baseline: 1.2296x; 1.0060x over previous
"""Trainium2 Bass kernel for nn_EvidentialGSL (8-core row-sharded), v2.

Same algorithm as the baseline kernel (see kernel.py docstring) with the
hot-loop restructured around DMA-issue cost and SBUF reuse:
  - every multi-tile load/store is a single 2/3-dim-AP DMA (HWDGE fixed
    overhead is ~625ns per dma_start, independent of size)
  - phase A processes i-tiles in pairs so the X^T hi/lo stream is read
    4x instead of 8x; A rows are DMA'd straight into the stripe and the
    relu(S) term is accumulated in place
  - V0T spill tiles are grouped [m=s%8][d=s//8] so phase B can fetch a
    strided octet (all j-tiles = o mod 8) with one DMA and start as soon
    as AllToAll #o lands
  - phase B element-wise work is split across DVE (compare/mult) and
    GpSimd (mask max)
"""
import os
import numpy as np
from contextlib import ExitStack

KPHASE = int(os.environ.get("KPHASE", "3"))

import ml_dtypes
from concourse import bass, bacc, tile, mybir
from concourse.bass_utils import run_bass_kernel_spmd

dt = mybir.dt
AF = mybir.ActivationFunctionType
ALU = mybir.AluOpType

N, D = 8192, 768
H1, H2 = 512, 256
NCORE = 8
P = N // NCORE          # 1024 rows per core
NIT = P // 128          # 8 i-tiles per core
NJT = N // 128          # 64 j-tiles
KD = D // 128           # 6
KH1 = H1 // 128         # 4
KH2 = H2 // 128         # 2
JC = 512                # phase-A j chunk
NJC = N // JC           # 16
NPAIR = NIT // 2        # 4 passes of i-tile pairs


def _softplus(nc, pool, out_ap, in_ap, shp, neg=False):
    t1 = pool.tile(shp, dt.float32, tag="sp_a")
    t2 = pool.tile(shp, dt.float32, tag="sp_b")
    nc.scalar.activation(t1[:], in_ap, AF.Abs)
    nc.scalar.activation(t1[:], t1[:], AF.Exp, scale=-1.0)
    nc.scalar.activation(t1[:], t1[:], AF.Ln, bias=1.0)
    nc.scalar.activation(t2[:], in_ap, AF.Relu, scale=(-1.0 if neg else 1.0))
    nc.vector.tensor_add(out_ap, t1[:], t2[:])


def _sigmoid(nc, pool, out_ap, in_ap, shp):
    t3 = pool.tile(shp, dt.float32, tag="sp_c")
    _softplus(nc, pool, t3[:], in_ap, shp, neg=True)
    nc.scalar.activation(out_ap, t3[:], AF.Exp, scale=-1.0)


def _head(nc, tc, psum, w1sb, b1sb, w2sb, b2sb, whsb, bhsb, xin, out_dram,
          obase, want_u0, hpool, addv):
    """Transposed NIG head on xin [128, KD, P] float32r; writes 4 output rows.

    Head matmuls run in f32r (1 cyc/row vs fp32's 4): ~1.6e-4 relative on the
    head outputs, well inside the 2e-2 budget and with no top-k sensitivity.
    """
    h1 = hpool.tile([128, KH1, P], dt.float32r, tag="h1t")
    for m in range(KH1):
        ps = psum.tile([128, P], dt.float32, tag="ph")
        for h in range(2):
            for k in range(KD):
                nc.tensor.matmul(ps[:, h * 512:(h + 1) * 512],
                                 w1sb[:, k, m * 128:(m + 1) * 128],
                                 xin[:, k, h * 512:(h + 1) * 512],
                                 start=(k == 0), stop=(k == KD - 1))
        nc.scalar.activation(h1[:, m, :], ps[:], AF.Gelu, bias=b1sb[:, m:m + 1])
    h2 = hpool.tile([128, KH2, P], dt.float32r, tag="h2t")
    for m in range(KH2):
        ps = psum.tile([128, P], dt.float32, tag="ph")
        for h in range(2):
            for k in range(KH1):
                nc.tensor.matmul(ps[:, h * 512:(h + 1) * 512],
                                 w2sb[:, k, m * 128:(m + 1) * 128],
                                 h1[:, k, h * 512:(h + 1) * 512],
                                 start=(k == 0), stop=(k == KH1 - 1))
        nc.scalar.activation(h2[:, m, :], ps[:], AF.Gelu, bias=b2sb[:, m:m + 1])
    ps4 = psum.tile([4, P], dt.float32, tag="ph")
    for h in range(2):
        for k in range(KH2):
            nc.tensor.matmul(ps4[:, h * 512:(h + 1) * 512], whsb[:, k, 0:4],
                             h2[:, k, h * 512:(h + 1) * 512],
                             start=(k == 0), stop=(k == KH2 - 1))
    r4 = hpool.tile([4, P], dt.float32, tag="r4")
    nc.scalar.activation(r4[:], ps4[:], AF.Identity, bias=bhsb[0:4, 0:1])
    nc.sync.dma_start(out=out_dram[obase:obase + 1, :], in_=r4[0:1, :])
    o1 = hpool.tile([4, P], dt.float32, tag="o4")
    _softplus(nc, hpool, o1[:], r4[:], [4, P])
    nc.vector.tensor_scalar(o1[:], o1[:], addv[0:4, 0:1], None, ALU.add)
    nc.sync.dma_start(out=out_dram[obase + 1:obase + 4, :], in_=o1[1:4, :])
    if not want_u0:
        return None
    a0t = hpool.tile([1, P], dt.float32, tag="a0t")
    b0t = hpool.tile([1, P], dt.float32, tag="b0t")
    nc.sync.dma_start(out=a0t[:], in_=o1[2:3, :])
    nc.sync.dma_start(out=b0t[:], in_=o1[3:4, :])
    nc.vector.tensor_scalar(a0t[:], a0t[:], -1.0, 1e-8, ALU.add, ALU.max)
    nc.vector.reciprocal(a0t[:], a0t[:])
    u0 = hpool.tile([1, P], dt.float32, tag="u0")
    nc.vector.tensor_mul(u0[:], b0t[:], a0t[:])
    return u0


def _head_h(nc, psum, w1sb, b1sb, w2sb, b2sb, whsb, bhsb, xin, out_dram,
            obase, hpool, addv, hh):
    """Half-width (512-col) variant of _head for one column half hh."""
    c0, c1 = hh * 512, (hh + 1) * 512
    h1 = hpool.tile([128, KH1, 512], dt.float32r, tag="h1h")
    for m in range(KH1):
        ps = psum.tile([128, 512], dt.float32, tag="ph")
        for k in range(KD):
            nc.tensor.matmul(ps[:], w1sb[:, k, m * 128:(m + 1) * 128],
                             xin[:, k, c0:c1], start=(k == 0), stop=(k == KD - 1))
        nc.scalar.activation(h1[:, m, :], ps[:], AF.Gelu, bias=b1sb[:, m:m + 1])
    h2 = hpool.tile([128, KH2, 512], dt.float32r, tag="h2h")
    for m in range(KH2):
        ps = psum.tile([128, 512], dt.float32, tag="ph")
        for k in range(KH1):
            nc.tensor.matmul(ps[:], w2sb[:, k, m * 128:(m + 1) * 128],
                             h1[:, k, :], start=(k == 0), stop=(k == KH1 - 1))
        nc.scalar.activation(h2[:, m, :], ps[:], AF.Gelu, bias=b2sb[:, m:m + 1])
    ps4 = psum.tile([4, 512], dt.float32, tag="ph")
    for k in range(KH2):
        nc.tensor.matmul(ps4[:], whsb[:, k, 0:4], h2[:, k, :],
                         start=(k == 0), stop=(k == KH2 - 1))
    r4 = hpool.tile([4, 512], dt.float32, tag="r4h")
    nc.scalar.activation(r4[:], ps4[:], AF.Identity, bias=bhsb[0:4, 0:1])
    nc.sync.dma_start(out=out_dram[obase:obase + 1, c0:c1], in_=r4[0:1, :])
    o1 = hpool.tile([4, 512], dt.float32, tag="o4h")
    _softplus(nc, hpool, o1[:], r4[:], [4, 512])
    nc.vector.tensor_scalar(o1[:], o1[:], addv[0:4, 0:1], None, ALU.add)
    nc.sync.dma_start(out=out_dram[obase + 1:obase + 4, c0:c1], in_=o1[1:4, :])



def build_nc(beta: float, gam: float, eps2: float):
    nc = bacc.Bacc("TRN2", target_bir_lowering=False, debug=False,
                   num_devices=NCORE)
    f32, f32r, bf16, u8 = dt.float32, dt.float32r, dt.bfloat16, dt.uint8

    # X reshaped [a][b][128][D] with j-tile jt = a*8 + b, so a strided octet
    # (fixed b) is a single 3D-AP DMA.
    X_d = nc.dram_tensor("X", [NIT, NCORE, 128, D], f32, kind="ExternalInput").ap()
    XTHI_d = nc.dram_tensor("XTHI", [KD, 128, N], bf16, kind="ExternalInput").ap()
    XTLO_d = nc.dram_tensor("XTLO", [KD, 128, N], bf16, kind="ExternalInput").ap()
    XMYT_d = nc.dram_tensor("XMYT", [KD, 128, P], f32, kind="ExternalInput").ap()
    AROW_d = nc.dram_tensor("AROW", [P, N], f32, kind="ExternalInput").ap()
    W_d = nc.dram_tensor("W_gm", [KD, 128, D], f32, kind="ExternalInput").ap()
    ihw1_d = nc.dram_tensor("ih_w1", [KD, 128, H1], f32, kind="ExternalInput").ap()
    ihb1_d = nc.dram_tensor("ih_b1", [KH1, 128], f32, kind="ExternalInput").ap()
    ihw2_d = nc.dram_tensor("ih_w2", [KH1, 128, H2], f32, kind="ExternalInput").ap()
    ihb2_d = nc.dram_tensor("ih_b2", [KH2, 128], f32, kind="ExternalInput").ap()
    ihwh_d = nc.dram_tensor("ih_wh", [KH2, 128, 4], f32, kind="ExternalInput").ap()
    ihbh_d = nc.dram_tensor("ih_bh", [4], f32, kind="ExternalInput").ap()
    gcnw_d = nc.dram_tensor("gcn_w", [KD, 128, D], f32, kind="ExternalInput").ap()
    gcnb_d = nc.dram_tensor("gcn_b", [KD, 128], f32, kind="ExternalInput").ap()
    fhw1_d = nc.dram_tensor("fh_w1", [KD, 128, H1], f32, kind="ExternalInput").ap()
    fhb1_d = nc.dram_tensor("fh_b1", [KH1, 128], f32, kind="ExternalInput").ap()
    fhw2_d = nc.dram_tensor("fh_w2", [KH1, 128, H2], f32, kind="ExternalInput").ap()
    fhb2_d = nc.dram_tensor("fh_b2", [KH2, 128], f32, kind="ExternalInput").ap()
    fhwh_d = nc.dram_tensor("fh_wh", [KH2, 128, 4], f32, kind="ExternalInput").ap()
    fhbh_d = nc.dram_tensor("fh_bh", [4], f32, kind="ExternalInput").ap()

    OUT_d = nc.dram_tensor("OUT", [8, P], f32, kind="ExternalOutput").ap()

    pid = nc.partition_id()
    groups = [list(range(NCORE))]

    with tile.TileContext(nc) as tc, ExitStack() as top:
        const = top.enter_context(tc.tile_pool(name="const", bufs=1))
        dram = top.enter_context(tc.tile_pool(name="dram", bufs=1, space="DRAM"))

        # V0T spill grouped [m = s%8][d = s//8][128][P]: consecutive-s write
        # batches are one 3D AP, strided-octet reads are one 3D AP.
        V0T_t = dram.tile([8, NIT, 128, P], f32)
        RSEND_t = dram.tile([NIT, NCORE, 128, P], u8)
        RRECV_t = dram.tile([NIT, NCORE, 128, P], u8)
        TMY_t = dram.tile([NIT, 128], f32)
        GD_t = dram.tile([1, P], f32)
        GALL_t = dram.tile([NCORE, P], f32)

        # ---- constants
        iota_i = const.tile([128, 128], dt.int32)
        nc.gpsimd.iota(iota_i[:], pattern=[[1, 128]], base=0, channel_multiplier=0)
        pidx_i = const.tile([128, 1], dt.int32)
        nc.gpsimd.iota(pidx_i[:], pattern=[[0, 1]], base=0, channel_multiplier=1)
        iota_f = const.tile([128, 128], f32)
        nc.vector.tensor_copy(iota_f[:], iota_i[:])
        pidx_f = const.tile([128, 1], f32)
        nc.vector.tensor_copy(pidx_f[:], pidx_i[:])
        eye = const.tile([128, 128], f32)
        nc.vector.tensor_scalar(eye[:], iota_f[:], pidx_f[:, 0:1], None, ALU.is_equal)
        ident = const.tile([128, 128], f32)
        nc.vector.tensor_copy(ident[:], eye[:])
        ones1 = const.tile([1, 128], f32)
        nc.vector.memset(ones1[:], 1.0)
        ones_f = const.tile([128, 1], f32)
        nc.vector.memset(ones_f[:], 1.0)
        ones_r = const.tile([128, 1], f32r)
        nc.vector.tensor_copy(ones_r[:], ones_f[:])
        addv = const.tile([128, 1], f32)
        nc.vector.tensor_scalar(addv[:], pidx_f[:], 2.0, None, ALU.is_equal)
        nc.vector.tensor_scalar(addv[:], addv[:], 1.0, 1e-6, ALU.mult, ALU.add)

        def load_kmaj(pool, dram_ap, kt, cols, dtype=f32, tag=None):
            t = pool.tile([128, kt, cols], dtype, tag=tag or f"w_{dram_ap.tensor.name}")
            nc.sync.dma_start(out=t[:], in_=dram_ap[:, :, :].bitcast(dtype)
                              .rearrange("k p c -> p k c"))
            return t

        def load_bias(pool, dram_ap, kt):
            tg = f"b_{dram_ap.tensor.name}"
            if kt == 0:
                t = pool.tile([4, 1], f32, tag=tg)
                nc.sync.dma_start(out=t[:, 0:1], in_=dram_ap[0:4])
            else:
                t = pool.tile([128, kt], f32, tag=tg)
                nc.sync.dma_start(out=t[:], in_=dram_ap[:, :].rearrange("k p -> p k"))
            return t

        t2rep = const.tile([128, P], f32)

        # ================= early phase: XWT, head1, G =================
        xw_stack = ExitStack()
        xwP = xw_stack.enter_context(tc.tile_pool(name="xwP", bufs=1))
        xwhi = xwP.tile([128, KD, P], bf16, tag="xwhi")
        xwlo = xwP.tile([128, KD, P], bf16, tag="xwlo")
        with tc.tile_pool(name="early", bufs=1) as early, \
             tc.tile_pool(name="psE", bufs=1, space="PSUM") as psE:
            xmyt = early.tile([128, KD, P], f32)
            nc.sync.dma_start(out=xmyt[:],
                              in_=XMYT_d[:, :, :].rearrange("k p c -> p k c"))
            Wsb = load_kmaj(early, W_d, KD, D)
            for m in range(KD):
                ps = psE.tile([128, P], f32, tag="pxw")
                for h in range(2):
                    for k in range(KD):
                        nc.tensor.matmul(ps[:, h * 512:(h + 1) * 512],
                                         Wsb[:, k, m * 128:(m + 1) * 128],
                                         xmyt[:, k, h * 512:(h + 1) * 512],
                                         start=(k == 0), stop=(k == KD - 1))
                nc.scalar.activation(xwhi[:, m, :], ps[:], AF.Copy)
                nc.vector.tensor_sub(xwlo[:, m, :], ps[:], xwhi[:, m, :])

            ihw1 = load_kmaj(early, ihw1_d, KD, H1, dt.float32r)
            ihw2 = load_kmaj(early, ihw2_d, KH1, H2, dt.float32r)
            ihwh = load_kmaj(early, ihwh_d, KH2, 4, dt.float32r)
            xmyt_r = early.tile([128, KD, P], dt.float32r, tag="xmyt_r")
            nc.sync.dma_start(out=xmyt_r[:],
                              in_=XMYT_d[:, :, :].bitcast(dt.float32r)
                              .rearrange("k p c -> p k c"))
            ihb1 = load_bias(early, ihb1_d, KH1)
            ihb2 = load_bias(early, ihb2_d, KH2)
            ihbh = load_bias(early, ihbh_d, 0)
            with tc.tile_pool(name="hpool", bufs=1) as hpool, \
                 tc.tile_pool(name="psE2", bufs=2, space="PSUM") as psE2:
                u0 = _head(nc, tc, psE2, ihw1, ihb1, ihw2, ihb2, ihwh, ihbh,
                           xmyt_r, OUT_d, 0, True, hpool, addv)
                sg = hpool.tile([1, P], f32, tag="sg")
                _sigmoid(nc, hpool, sg[:], u0[:], [1, P])
                gmy = hpool.tile([1, P], f32, tag="gmy")
                nc.vector.tensor_scalar(gmy[:], sg[:], float(np.float32(-gam)),
                                        1.0, ALU.mult, ALU.add)
                nc.sync.dma_start(out=GD_t[0:1, :], in_=gmy[0:1, :])
                nc.gpsimd.collective_compute("AllGather", ALU.bypass,
                                             replica_groups=groups,
                                             ins=[GD_t.opt()], outs=[GALL_t.opt()])

        # ================= phase A =================
        NPAIR_RUN = NPAIR if KPHASE != 0 else 1
        with tc.tile_pool(name="stripeP", bufs=3) as stripeP, \
             tc.tile_pool(name="pa", bufs=3) as pa, \
             tc.tile_pool(name="pam", bufs=2) as pam, \
             tc.tile_pool(name="pam1", bufs=1) as pam1, \
             tc.tile_pool(name="psA", bufs=2, space="PSUM") as psA, \
             tc.tile_pool(name="psT", bufs=4, space="PSUM") as psT:
            for pr in range(NPAIR_RUN):
                stripes = []
                for i01 in range(2):
                    st = stripeP.tile([128, N], f32, tag="v0")
                    stripes.append(st)
                for jc in range(NJC):
                    xh = pa.tile([128, KD, JC], bf16, tag="xth")
                    xl = pa.tile([128, KD, JC], bf16, tag="xtl")
                    nc.sync.dma_start(
                        out=xh[:], in_=XTHI_d[:, :, jc * JC:(jc + 1) * JC]
                        .rearrange("k p c -> p k c"))
                    nc.sync.dma_start(
                        out=xl[:], in_=XTLO_d[:, :, jc * JC:(jc + 1) * JC]
                        .rearrange("k p c -> p k c"))
                    if jc == 0:
                        # A-row loads issue after the first X^T chunk so the
                        # 8MB transfer does not delay the first matmuls
                        for i01 in range(2):
                            it = pr * 2 + i01
                            nc.sync.dma_start(
                                out=stripes[i01][:],
                                in_=AROW_d[it * 128:(it + 1) * 128, :])
                    for i01 in range(2):
                        it = pr * 2 + i01
                        ps = psA.tile([128, JC], f32, tag=f"psv{i01}")
                        first = True
                        for pi, (aa, bb) in enumerate(
                                ((xwhi, xh), (xwhi, xl), (xwlo, xh))):
                            for k in range(KD):
                                nc.tensor.matmul(
                                    ps[:], aa[:, k, it * 128:(it + 1) * 128],
                                    bb[:, k, :],
                                    start=first, stop=(pi == 2 and k == KD - 1))
                                first = False
                        rel = pa.tile([128, JC], f32, tag=f"rel{i01}")
                        nc.scalar.activation(rel[:], ps[:], AF.Relu,
                                             scale=float(np.float32(1.0 / beta)))
                        sl = stripes[i01][:, jc * JC:(jc + 1) * JC]
                        nc.gpsimd.tensor_add(sl, sl, rel[:])
                for i01 in range(2):
                    it = pr * 2 + i01
                    stripe = stripes[i01]
                    top8 = pam.tile([128, 8], f32, tag="top8")
                    nc.vector.max(top8[:], stripe[:])
                    nc.sync.dma_start(out=TMY_t[it:it + 1, :], in_=top8[:, 4:5])
                    off = nc.snap(pid * P + it * 128, min_val=0, max_val=N - 128)
                    dsub = stripe[:, bass.ds(off, 128)]
                    nc.vector.scalar_tensor_tensor(dsub, eye[:], -1e9, dsub,
                                                   ALU.mult, ALU.add)
                    rmask = pam1.tile([128, N], u8, tag="rmask")
                    nc.vector.tensor_scalar(rmask[:], stripe[:], top8[:, 4:5], None,
                                            ALU.is_ge)
                    nc.sync.dma_start(
                        out=RSEND_t[it].rearrange("c p j -> p c j"), in_=rmask[:])
                    for d8 in range(NIT):
                        ct = pa.tile([128, 8, 128], f32, tag="ctr")
                        for m8 in range(8):
                            s = d8 * 8 + m8
                            pst = psT.tile([128, 128], f32, tag="ptr")
                            nc.tensor.transpose(pst[:], stripe[:, s * 128:(s + 1) * 128],
                                                ident[:])
                            nc.scalar.activation(ct[:, m8, :], pst[:], AF.Copy)
                        nc.sync.dma_start(
                            out=V0T_t[:, d8, :, it * 128:(it + 1) * 128]
                            .rearrange("m p c -> p m c"),
                            in_=ct[:])
                    nc.gpsimd.collective_compute(
                        "AllToAll", ALU.bypass, replica_groups=groups,
                        ins=[RSEND_t[it].opt()], outs=[RRECV_t[it].opt()])


        # T2rep broadcast (exact fp32 K=1 matmul)
        trow = const.tile([1, P], f32)
        nc.sync.dma_start(out=trow[0:1, :], in_=TMY_t[:])
        if KPHASE >= 2:
          with tc.tile_pool(name="psB1", bufs=1, space="PSUM") as psB1:
            for h in range(2):
                psb = psB1.tile([128, 512], f32, tag="pbc")
                nc.tensor.matmul(psb[:], ones1[:], trow[0:1, h * 512:(h + 1) * 512],
                                 start=True, stop=True)
                nc.scalar.activation(t2rep[:, h * 512:(h + 1) * 512], psb[:], AF.Copy)

        # ================= phase B =================
        xw_stack.close()
        if KPHASE >= 2:
            bc = top.enter_context(tc.tile_pool(name="bc", bufs=1))
            pt_acc = bc.tile([128, KD, P], f32, tag="pt_acc")
            rs_acc = bc.tile([1, P], f32, tag="rs_acc")
            gcnw = load_kmaj(bc, gcnw_d, KD, D, f32r)
            gcnb = load_bias(bc, gcnb_d, KD)
            with tc.tile_pool(name="pb", bufs=1) as pb, \
                 tc.tile_pool(name="pbm", bufs=2) as pbm, \
                 tc.tile_pool(name="pbt", bufs=1) as pbt, \
                 tc.tile_pool(name="agtP", bufs=1) as agtP, \
                 tc.tile_pool(name="psP", bufs=1, space="PSUM") as psP, \
                 tc.tile_pool(name="psR", bufs=1, space="PSUM") as psR:
                for o in range(8):
                    # strided octet: j-tiles jt = o + 8*l for l = 0..7.
                    # bufs=1 pools with per-half tags: octet o+1's first-half
                    # load overlaps octet o's second-half compute.
                    # quarter-granular staggered loads: finer tile release
                    # lets octet o+1's first quarter start while o drains
                    v0t, xt_, rcv = [], [], []
                    for half in range(4):
                        vt = pb.tile([128, 2, P], f32, tag=f"v0t{half}")
                        nc.sync.dma_start(
                            out=vt[:],
                            in_=V0T_t[o, half * 2:(half + 1) * 2]
                            .rearrange("d p c -> p d c"))
                        v0t.append(vt)
                        rc = pb.tile([128, 2, P], u8, tag=f"rcv{half}")
                        nc.sync.dma_start(
                            out=rc[:],
                            in_=RRECV_t[o, half * 2:(half + 1) * 2]
                            .rearrange("c p i -> p c i"))
                        rcv.append(rc)
                        xt = pb.tile([128, 2, D], f32, tag=f"xrow{half}")
                        nc.sync.dma_start(
                            out=xt[:],
                            in_=X_d[half * 2:(half + 1) * 2, o]
                            .rearrange("a p d -> p a d"))
                        xt_.append(xt)
                    gsl = pbm.tile([128, NCORE], f32, tag="gsl")
                    nc.sync.dma_start(
                        out=gsl[:],
                        in_=GALL_t[:, o * 128:(o + 1) * 128].rearrange("l p -> p l"))
                    agts, xgs = [], []
                    for l in range(8):
                        vt = v0t[l // 2][:, l % 2, :]
                        mlt = pbm.tile([128, P], u8, tag="mlt")
                        nc.vector.tensor_tensor(mlt[:], vt, t2rep[:], ALU.is_ge)
                        msk = pbm.tile([128, P], u8, tag="msk")
                        nc.vector.tensor_tensor(msk[:], mlt[:],
                                                rcv[l // 2][:, l % 2, :], ALU.max)
                        agt = agtP.tile([128, P], f32r, tag=f"agt{l}")
                        nc.vector.tensor_tensor(agt[:], vt, msk[:], ALU.mult)
                        agts.append(agt)
                        xg = agtP.tile([128, D], f32r, tag=f"xg{l}")
                        nc.scalar.activation(xg[:], xt_[l // 2][:, l % 2, :],
                                             AF.Copy, scale=gsl[:, l:l + 1])
                        xgs.append(xg)
                    for h in range(2):
                        pp = psP.tile([128, KD, 512], f32, tag="pp")
                        for l in range(8):
                            for m in range(KD):
                                nc.tensor.matmul(pp[:, m, :],
                                                 xgs[l][:, m * 128:(m + 1) * 128],
                                                 agts[l][:, h * 512:(h + 1) * 512],
                                                 start=(l == 0), stop=(l == 7))
                        # drain PSUM via ACT, accumulate on Pool: keeps DVE
                        # free for the mask/agt chain
                        if o == 0:
                            for m in range(KD):
                                nc.scalar.activation(
                                    pt_acc[:, m, h * 512:(h + 1) * 512],
                                    pp[:, m, :], AF.Copy)
                        else:
                            ptmp = pbt.tile([128, KD, 512], f32, tag="ptmp")
                            for m in range(KD):
                                nc.scalar.activation(ptmp[:, m, :], pp[:, m, :],
                                                     AF.Copy)
                            for m in range(KD):
                                nc.gpsimd.tensor_add(
                                    pt_acc[:, m, h * 512:(h + 1) * 512],
                                    pt_acc[:, m, h * 512:(h + 1) * 512],
                                    ptmp[:, m, :])
                    for h in range(2):
                        pr2 = psR.tile([1, 512], f32, tag="pr")
                        for l in range(8):
                            nc.tensor.matmul(pr2[0:1, :],
                                             ones_r[:, 0:1],
                                             agts[l][:, h * 512:(h + 1) * 512],
                                             start=(l == 0), stop=(l == 7))
                        if o == 0:
                            nc.vector.tensor_copy(rs_acc[0:1, h * 512:(h + 1) * 512],
                                                  pr2[:])
                        else:
                            nc.vector.tensor_add(rs_acc[0:1, h * 512:(h + 1) * 512],
                                                 rs_acc[0:1, h * 512:(h + 1) * 512],
                                                 pr2[:])

        # ================= phase C =================
        if KPHASE >= 3:
            with tc.tile_pool(name="pc", bufs=1) as pc, \
                 tc.tile_pool(name="hpool2", bufs=1) as hpool2, \
                 tc.tile_pool(name="psC", bufs=1, space="PSUM") as psC, \
                 tc.tile_pool(name="psCh", bufs=2, space="PSUM") as psCh:
                fhw1 = load_kmaj(pc, fhw1_d, KD, H1, f32r)
                fhw2 = load_kmaj(pc, fhw2_d, KH1, H2, f32r)
                fhwh = load_kmaj(pc, fhwh_d, KH2, 4, f32r)
                fhb1 = load_bias(pc, fhb1_d, KH1)
                fhb2 = load_bias(pc, fhb2_d, KH2)
                fhbh = load_bias(pc, fhbh_d, 0)
                pt_acc_r = pc.tile([128, KD, P], f32r, tag="pt_acc_r")
                nc.vector.tensor_copy(pt_acc_r[:], pt_acc[:])
                dinv = pc.tile([1, P], f32, tag="dinv")
                nc.vector.tensor_scalar(dinv[:], rs_acc[:], float(np.float32(eps2)),
                                        None, ALU.max)
                nc.vector.reciprocal(dinv[:], dinv[:])
                drep = pc.tile([128, P], f32)
                psb = psC.tile([128, P], f32, tag="pbc")
                for h in range(2):
                    nc.tensor.matmul(psb[:, h * 512:(h + 1) * 512], ones1[:],
                                     dinv[0:1, h * 512:(h + 1) * 512],
                                     start=True, stop=True)
                nc.scalar.activation(drep[:], psb[:], AF.Copy)

                xmyt = pc.tile([128, KD, P], f32, tag="xmyt2")
                nc.sync.dma_start(out=xmyt[:],
                                  in_=XMYT_d[:, :, :].rearrange("k p c -> p k c"))

                # column-half pipeline: the fh head on half 0 overlaps the
                # gcn/gelu production of half 1
                xpm = pc.tile([128, KD, P], f32r)
                for hh in range(2):
                    c0, c1 = hh * 512, (hh + 1) * 512
                    for m in range(KD):
                        ps = psC.tile([128, 512], f32, tag="pxw")
                        for k in range(KD):
                            nc.tensor.matmul(ps[:],
                                             gcnw[:, k, m * 128:(m + 1) * 128],
                                             pt_acc_r[:, k, c0:c1],
                                             start=(k == 0), stop=(k == KD - 1))
                        tmp = pc.tile([128, 512], f32, tag="mtmp")
                        nc.vector.tensor_mul(tmp[:], ps[:], drep[:, c0:c1])
                        mf = pc.tile([128, 512], f32, tag="mf")
                        nc.scalar.activation(mf[:], tmp[:], AF.Gelu,
                                             bias=gcnb[:, m:m + 1])
                        nc.vector.tensor_add(xpm[:, m, c0:c1], xmyt[:, m, c0:c1],
                                             mf[:])
                    _head_h(nc, psCh, fhw1, fhb1, fhw2, fhb2, fhwh, fhbh,
                            xpm, OUT_d, 4, hpool2, addv, hh)

    nc.finalize()
    return nc


_NC_CACHE = {}
_last_in_maps = None


def kernel(**inputs) -> tuple:
    X = np.ascontiguousarray(np.asarray(inputs["X"], dtype=np.float32))
    A = np.asarray(inputs["A"], dtype=np.float32)
    ra = float(np.asarray(inputs["ra"], dtype=np.float64))
    gam = float(np.asarray(inputs["gam"], dtype=np.float64))
    al = float(np.float32(1.0) / (np.float32(1.0) + np.float32(np.exp(-np.float32(ra)))))
    beta = al / (1.0 - al)
    eps2 = 1e-8 / al

    XT = np.ascontiguousarray(X.T)
    XTHI = XT.astype(ml_dtypes.bfloat16)
    XTLO = (XT - XTHI.astype(np.float32)).astype(ml_dtypes.bfloat16)

    key = (round(beta, 12), round(gam, 12), KPHASE)
    if key not in _NC_CACHE:
        _NC_CACHE[key] = build_nc(beta, gam, eps2)
    nc = _NC_CACHE[key]

    rep = {
        "X": X.reshape(NIT, NCORE, 128, D),
        "XTHI": XTHI.reshape(KD, 128, N),
        "XTLO": XTLO.reshape(KD, 128, N),
        "W_gm": None, "gcn_w": None,
    }
    for k, kt, cols in (("W_gm", KD, D), ("ih_w1", KD, H1), ("ih_w2", KH1, H2),
                        ("ih_wh", KH2, 4), ("gcn_w", KD, D), ("fh_w1", KD, H1),
                        ("fh_w2", KH1, H2), ("fh_wh", KH2, 4)):
        rep[k] = np.ascontiguousarray(
            np.asarray(inputs[k], dtype=np.float32)).reshape(kt, 128, cols)
    for k, kt in (("ih_b1", KH1), ("ih_b2", KH2), ("gcn_b", KD),
                  ("fh_b1", KH1), ("fh_b2", KH2)):
        rep[k] = np.ascontiguousarray(
            np.asarray(inputs[k], dtype=np.float32)).reshape(kt, 128)
    for k in ("ih_bh", "fh_bh"):
        rep[k] = np.ascontiguousarray(np.asarray(inputs[k], dtype=np.float32))

    in_maps = []
    for c in range(NCORE):
        m = dict(rep)
        m["XMYT"] = np.ascontiguousarray(XT[:, c * P:(c + 1) * P]).reshape(KD, 128, P)
        m["AROW"] = np.ascontiguousarray(A[c * P:(c + 1) * P, :])
        in_maps.append(m)

    global _last_in_maps
    _last_in_maps = in_maps
    res = run_bass_kernel_spmd(nc, in_maps, list(range(NCORE)))
    full = np.concatenate([res.results[c]["OUT"] for c in range(NCORE)], axis=1)
    return tuple(full[i] for i in range(8))


if __name__ == "__main__":
    import jax
    import reference
    cpu = jax.devices("cpu")[0]
    with jax.default_device(cpu):
        inp = reference.setup_inputs()
        inp = {k: np.asarray(v) for k, v in inp.items()}
    got = kernel(**inp)
    with jax.default_device(cpu):
        exp = [np.asarray(x) for x in reference.reference(
            **{k: jax.device_put(v, cpu) for k, v in inp.items()})]
    for i, (g, e) in enumerate(zip(got, exp)):
        e = np.asarray(e)
        err = np.abs(g - e).max()
        rel = err / max(np.abs(e).max(), 1e-9)
        print(f"out{i}: maxabs {err:.3e} rel {rel:.3e}")
